# revision 33
# baseline (speedup 1.0000x reference)
"""Distributed Bass kernel: LN + multi-head ALiBi attention + out-proj.

Sharding: 8 cores = (batch b in 0..3) x (query-token half t in 0..1).
Each core computes the full pipeline for its 1024 query tokens (all 16
heads); K/V are computed for the full 2048-token sequence (duplicated
across the 2 cores of a batch).  No collectives.

SPMD trick: every core runs the SAME graph.  Core (b, t) receives x[b]
rolled by -1024*t along tokens, so its query tokens always sit at local
rows 0..1023.  The ALiBi distance table ("master") is per-core DATA
encoding true global distances (two planes: j-tiles < 8 and >= 8, which
for t=1 differ by a 2048 wrap).  Blocks a core computes needlessly are
killed by the bias (exp(-large) ~ 0).

ALiBi banding: head h only effectively attends within |i-j| <= T_h =
ceil(23/s_h) (dropped softmax mass < ~1e-7 of the denominator), so
score blocks outside the band are skipped statically.

The per-head 1/(8*s_h) is folded into Wq on the host; the ALiBi bias is
added on DVE (PSUM + master slice), and ACT exp applies scale=s_h, so
exp(s_h*(QK/(8 s_h) + master)) is the softmax numerator.  The softmax
denominator arrives as a ones-column of the PV matmul; per q-chunk all
16 heads' sums are gathered into one PSUM tile via one-hot matmuls so a
SINGLE Ln serves the chunk (avoids ACT table-set thrashing), then a
per-head ones-matmul broadcast + exp(-x) forms 1/l across partitions
and OT is normalized in place.
"""

import os
import sys

sys.path.insert(0, "/opt/trn_rl_repo")

import numpy as np
import ml_dtypes

import concourse.bass as bass
import concourse.mybir as mybir
import concourse.tile as tile
from concourse import bacc
from concourse.bass import ts
from concourse.bass_utils import run_bass_kernel_spmd

BF16 = mybir.dt.bfloat16
F32 = mybir.dt.float32
F32R = mybir.dt.float32r

CTX = 2048
DIM = 1024
NH = 16
DH = 64
QTOK = 1024  # query tokens per core
EPS = 1e-5
MW = 3072  # master table width

LAST_EXEC_NS = None


def _band_blocks(T, qc):
    """j-tile list for query chunk qc (local g0 = qc*512), band half-width T."""
    g0 = qc * 512
    lo = max(0, g0 - T) // 128
    hi = (min(CTX, g0 + 512 + T) + 127) // 128
    jts = set(range(lo, hi))
    if qc == 0 and T < CTX:
        # wrap blocks: j_local in [2048-T, 2048) carries the left band of the
        # t=1 core (j_global ~ 1024-T..1024); bias-killed garbage for t=0.
        jts |= set(range((CTX - T) // 128, CTX // 128))
    return sorted(jts)


def _build_graph(s_heads, Ts):
    """Build the shared SPMD Bass graph; returns compiled nc."""
    nc = bacc.Bacc("TRN2", target_bir_lowering=False, debug=False)

    x_d = nc.dram_tensor("x", [CTX, DIM], F32, kind="ExternalInput").ap()
    wq_d = nc.dram_tensor("wq", [8, 128, 8, 128], BF16, kind="ExternalInput").ap()
    wk_d = nc.dram_tensor("wk", [8, 128, 8, 128], BF16, kind="ExternalInput").ap()
    wv_d = nc.dram_tensor("wv", [8, 128, DIM], BF16, kind="ExternalInput").ap()
    wo_d = nc.dram_tensor("wo", [8, 128, DIM], BF16, kind="ExternalInput").ap()
    mst_d = nc.dram_tensor("master", [128, MW], BF16, kind="ExternalInput").ap()
    mst1_d = nc.dram_tensor("master1", [128, 2048], BF16, kind="ExternalInput").ap()
    one_d = nc.dram_tensor("ones", [1, 128], F32R, kind="ExternalInput").ap()
    idn_d = nc.dram_tensor("ident", [128, 128], BF16, kind="ExternalInput").ap()
    eye_d = nc.dram_tensor("eyerow", [1, 256], F32R, kind="ExternalInput").ap()
    oh_d = nc.dram_tensor("oh16", [16, 2048], F32R, kind="ExternalInput").ap()
    out_d = nc.dram_tensor("out", [QTOK, DIM], F32, kind="ExternalOutput").ap()

    AF = mybir.ActivationFunctionType
    ALU = mybir.AluOpType

    with tile.TileContext(nc) as tc:
        with (
            tc.tile_pool(name="persist", bufs=1) as pp,
            tc.tile_pool(name="dram", bufs=1, space="DRAM") as dp,
            tc.tile_pool(name="xio", bufs=2) as xp,
            tc.tile_pool(name="xnp", bufs=4) as xnp,
            tc.tile_pool(name="wstream", bufs=3) as wp,
            tc.tile_pool(name="ptile", bufs=2) as ptp,
            tc.tile_pool(name="small", bufs=4) as sp,
            tc.tile_pool(name="small2", bufs=2) as sp2,
            tc.tile_pool(name="norm", bufs=2) as epool,
            tc.tile_pool(name="wopool", bufs=1) as wop,
            tc.tile_pool(name="opool", bufs=2) as op,
            tc.tile_pool(name="ps_proj", bufs=2, space="PSUM") as ps_proj,
            tc.tile_pool(name="ps_s", bufs=2, space="PSUM") as ps_s,
            tc.tile_pool(name="ps_o", bufs=2, space="PSUM") as ps_o,
            tc.tile_pool(name="ps_n", bufs=1, space="PSUM") as ps_n,
        ):
            # ---- persistent SBUF ----
            master = pp.tile([128, MW], BF16, tag="master")
            master1 = pp.tile([128, 2048], BF16, tag="master1")
            ones = pp.tile([1, 128], F32R, tag="ones")
            ident = pp.tile([128, 128], BF16, tag="ident")
            eyer = pp.tile([1, 256], F32R, tag="eyer")
            oh16 = pp.tile([16, 2048], F32R, tag="oh16")
            xnT = pp.tile([128, 8, CTX], BF16, tag="big")  # slot shared with OT
            KT = pp.tile([128, 8, CTX], BF16, tag="KT")
            QT = pp.tile([128, 8, QTOK], BF16, tag="QT")
            Vsb = pp.tile([128, 16, NH, 65], BF16, tag="Vsb")
            wv_sb = pp.tile([128, 8, DIM], BF16, tag="wv")

            nc.sync.dma_start(master[:], mst_d[:])
            nc.sync.dma_start(master1[:], mst1_d[:])
            nc.sync.dma_start(ones[:], one_d[:])
            nc.sync.dma_start(ident[:], idn_d[:])
            nc.sync.dma_start(eyer[:], eye_d[:])
            nc.sync.dma_start(oh16[:], oh_d[:])
            eps_sb = pp.tile([128, 1], F32, tag="eps")
            nc.any.memset(eps_sb[:], EPS)
            warm = pp.tile([1, 1], F32, tag="warm")
            nc.scalar.activation(warm[:], eps_sb[0:1, :], AF.Ln, bias=eps_sb[0:1, :])
            nc.scalar.activation(warm[:], warm[:], AF.Exp)
            nc.any.memset(Vsb[:, :, :, 64:65], 1.0)
            nc.sync.dma_start(wv_sb[:], wv_d.rearrange("k p d -> p k d"))

            # ---- Phases A+B+C pipelined per 512-token chunk ----
            for ch in range(4):
                mv4 = sp.tile([128, 4, 2], F32, tag="mv4")
                for i, tt in enumerate(range(4 * ch, 4 * ch + 4)):
                    xt = xp.tile([128, DIM], F32, tag="xt")
                    nc.sync.dma_start(xt[:], x_d[ts(tt, 128), :])
                    st6 = sp.tile([128, 2, 6], F32, tag="st6")
                    nc.vector.bn_stats(st6[:, 0, :], xt[:, 0:512])
                    nc.vector.bn_stats(st6[:, 1, :], xt[:, 512:1024])
                    nc.vector.bn_aggr(mv4[:, i, :], st6[:])
                # rstd for 4 tiles in one Ln + one Exp (keeps table set put)
                lv4 = sp.tile([128, 4], F32, tag="lv4")
                nc.scalar.activation(lv4[:], mv4[:, :, 1], AF.Ln, bias=eps_sb[:])
                rs4 = sp.tile([128, 4], F32, tag="rs4")
                nc.scalar.activation(rs4[:], lv4[:], AF.Exp, scale=-0.5)
                xns = []
                for i, tt in enumerate(range(4 * ch, 4 * ch + 4)):
                    xt = xp.tile([128, DIM], F32, tag="xt")
                    nc.sync.dma_start(xt[:], x_d[ts(tt, 128), :])
                    xn = xnp.tile([128, DIM], BF16, tag="xn")
                    nc.vector.tensor_scalar(
                        xn[:], xt[:], mv4[:, i, 0:1], rs4[:, i:i + 1],
                        ALU.subtract, ALU.mult,
                    )
                    xns.append(xn)
                for ko in range(8):  # transpose on PE (proj PSUM pool reused)
                    tp = ps_proj.tile([128, 512], F32, tag="pp")
                    tpv = tp[:].bitcast(BF16)
                    for i in range(4):
                        nc.tensor.transpose(
                            tpv[:, ts(i, 128)], xns[i][:, ts(ko, 128)], ident[:]
                        )
                    nc.any.tensor_copy(xnT[:, ko, ts(ch, 512)], tpv[:, 0:512])
                for dqt in range(8):  # K^T chunk
                    wkt = wp.tile([128, 8, 128], BF16, tag="wqt")
                    nc.scalar.dma_start(wkt[:], wk_d[dqt])
                    ps = ps_proj.tile([128, 512], F32, tag="pp")
                    for ko in range(8):
                        nc.tensor.matmul(
                            ps[:], wkt[:, ko, :], xnT[:, ko, ts(ch, 512)],
                            start=(ko == 0), stop=(ko == 7),
                        )
                    nc.vector.tensor_copy(KT[:, dqt, ts(ch, 512)], ps[:])
                if ch < 2:
                    for dqt in range(8):  # Q^T chunk
                        wqt = wp.tile([128, 8, 128], BF16, tag="wqt")
                        nc.scalar.dma_start(wqt[:], wq_d[dqt])
                        ps = ps_proj.tile([128, 512], F32, tag="pp")
                        for ko in range(8):
                            nc.tensor.matmul(
                                ps[:], wqt[:, ko, :], xnT[:, ko, ts(ch, 512)],
                                start=(ko == 0), stop=(ko == 7),
                            )
                        nc.vector.tensor_copy(QT[:, dqt, ts(ch, 512)], ps[:])
                for jt in range(4 * ch, 4 * ch + 4):  # V chunk
                    for dvc in range(2):
                        ps = ps_proj.tile([128, 512], F32, tag="pp")
                        for ko in range(8):
                            nc.tensor.matmul(
                                ps[:], xnT[:, ko, ts(jt, 128)],
                                wv_sb[:, ko, ts(dvc, 512)],
                                start=(ko == 0), stop=(ko == 7),
                            )
                        nc.vector.tensor_copy(
                            Vsb[:, jt, dvc * 8:(dvc + 1) * 8, 0:64],
                            ps[:].rearrange("p (h d) -> p h d", d=64),
                        )

            # OT shares the xnT slot (all xnT readers are above)
            OT = pp.tile([128, 8, QTOK], BF16, tag="big")

            # ---- Phase D: banded attention (qc outer; batched softmax norm) --
            # Heads are processed in pairs occupying PE row-groups [0:64] and
            # [64:128]; their score matmuls run concurrently on the array.
            head_order = list(range(NH))
            head_groups = [(h,) for h in head_order]
            for qc in range(QTOK // 512):
                lcol = ps_n.tile([16, 512], F32, tag="lcol")
                for grp_heads in head_groups:
                    blocks = _band_blocks(max(Ts[h] for h in grp_heads), qc)
                    nb = len(blocks)
                    po = {}
                    for h in grp_heads:
                        po[h] = ps_o.tile([65, 512], F32, tag="po", name=f"po{h}")
                    for b0 in range(0, nb, 2):
                        grp = blocks[b0:b0 + 2]
                        g = len(grp)
                        ein = {}
                        pt = {}
                        for h in grp_heads:
                            ein[h] = ptp.tile([128, 1024], F32, tag="ein", name=f"ein{h}")
                            pt[h] = ptp.tile([128, 1024], BF16, tag="pt", name=f"pt{h}")
                        for gi, jt in enumerate(grp):
                            pss = {}
                            for h in grp_heads:
                                base = 64 * (h % 2)
                                dqt = h // 2
                                pss[h] = ps_s.tile([128, 512], F32, tag="ps", name=f"pss{h}")
                                nc.tensor.matmul(
                                    pss[h][:],
                                    KT[base:base + 64, dqt, ts(jt, 128)],
                                    QT[base:base + 64, dqt, ts(qc, 512)],
                                    start=True, stop=True,
                                )
                            off = 2048 + 512 * qc - 128 * jt
                            msrc = master1 if jt >= 8 else master
                            for h in grp_heads:
                                nc.vector.tensor_tensor(
                                    ein[h][:, ts(gi, 512)], pss[h][:],
                                    msrc[:, off:off + 512], ALU.add,
                                )
                        for h in grp_heads:
                            nc.scalar.activation(
                                pt[h][:, 0:g * 512], ein[h][:, 0:g * 512],
                                AF.Exp, scale=float(s_heads[h]),
                            )
                        for gi, jt in enumerate(grp):
                            for h in grp_heads:
                                nc.tensor.matmul(
                                    po[h][:], Vsb[:, jt, h, :],
                                    pt[h][:, ts(gi, 512)],
                                    start=(b0 + gi == 0),
                                    stop=(b0 + gi == nb - 1),
                                )
                    for h in grp_heads:
                        base = 64 * (h % 2)
                        dqt = h // 2
                        # gather this head's softmax sums into the shared lcol
                        lt = sp2.tile([1, 512], F32R, tag="ltmp")
                        nc.scalar.copy(lt[:], po[h][64:65, :])
                        nc.tensor.matmul(
                            lcol[:], eyer[0:1, ts(h, 16)], lt[:],
                            start=(h == head_order[0]),
                            stop=(h == head_order[-1]),
                        )
                        # stash unnormalized O^T
                        if base == 0:
                            nc.any.tensor_copy(
                                OT[0:64, dqt, ts(qc, 512)], po[h][0:64, :]
                            )
                        else:
                            tmp = epool.tile([64, 512], BF16, tag="otmp")
                            nc.any.tensor_copy(tmp[:], po[h][0:64, :])
                            nc.sync.dma_start(
                                OT[64:128, dqt, ts(qc, 512)], tmp[:]
                            )
                # batched softmax normalization: one Ln for all 16 heads
                lnl = sp2.tile([16, 512], F32R, tag="lnl16")
                nc.scalar.activation(lnl[:], lcol[:], AF.Ln)
                for h in range(NH):
                    base = 64 * (h % 2)
                    dqt = h // 2
                    pb = ps_n.tile([128, 512], F32, tag="pb")
                    nc.tensor.matmul(
                        pb[:], oh16[:, ts(h, 128)], lnl[:],
                        start=True, stop=True,
                    )
                    einv = epool.tile([128, 512], BF16, tag="einv")
                    nc.scalar.activation(einv[:], pb[:], AF.Exp, scale=-1.0)
                    nc.vector.tensor_tensor(
                        OT[base:base + 64, dqt, ts(qc, 512)],
                        OT[base:base + 64, dqt, ts(qc, 512)],
                        einv[base:base + 64, :], ALU.mult,
                    )
                # ---- output projection for this q-half (overlaps next qc) ----
                for ec in range(2):
                    wot = wop.tile([128, 8, 512], BF16, tag="wot")
                    nc.sync.dma_start(
                        wot[:], wo_d[:, :, ts(ec, 512)].rearrange("h p e -> p h e")
                    )
                    for it in range(4 * qc, 4 * qc + 4):
                        ps = ps_proj.tile([128, 512], F32, tag="pp")
                        for hdt in range(8):
                            nc.tensor.matmul(
                                ps[:], OT[:, hdt, ts(it, 128)], wot[:, hdt, :],
                                start=(hdt == 0), stop=(hdt == 7),
                            )
                        ot = op.tile([128, 512], F32, tag="ot")
                        nc.any.tensor_copy(ot[:], ps[:])
                        nc.sync.dma_start(out_d[ts(it, 128), ts(ec, 512)], ot[:])

    nc.compile()
    return nc


def _prep(x, ln_w, ln_b, Wq, Wk, Wv, Wo, M):
    """Host-side input preparation -> (s_heads, Ts, in_maps)."""
    x = np.asarray(x, np.float32)
    ln_w = np.asarray(ln_w, np.float32)
    ln_b = np.asarray(ln_b, np.float32)
    Wq = np.asarray(Wq, np.float32)
    Wk = np.asarray(Wk, np.float32)
    Wv = np.asarray(Wv, np.float32)
    Wo = np.asarray(Wo, np.float32)
    M = np.asarray(M, np.float32)
    assert not np.any(ln_b), "kernel assumes ln_b == 0 (folded LN bias unsupported)"

    s_heads = (-M[:, 0, 1]).astype(np.float64)  # M[h,0,1] = -s_h
    Ts = [min(CTX, int(np.ceil(23.0 / s))) for s in s_heads]

    wq_eff = ln_w[:, None] * Wq
    for h in range(NH):
        wq_eff[:, h * DH:(h + 1) * DH] /= 8.0 * s_heads[h]
    wk_eff = ln_w[:, None] * Wk
    wv_eff = ln_w[:, None] * Wv

    def wq_layout(w):  # [1024,1024] -> [dqt, p, ko, m]
        return np.ascontiguousarray(
            w.reshape(8, 128, 8, 128).transpose(2, 1, 0, 3)
        ).astype(ml_dtypes.bfloat16)

    wq_a = wq_layout(wq_eff)
    wk_a = wq_layout(wk_eff)
    wv_a = np.ascontiguousarray(wv_eff.reshape(8, 128, DIM)).astype(
        ml_dtypes.bfloat16
    )
    wo_a = np.ascontiguousarray(Wo.reshape(8, 128, DIM)).astype(ml_dtypes.bfloat16)

    ones = np.ones((1, 128), np.float32)
    ident = np.eye(128, dtype=np.float32).astype(ml_dtypes.bfloat16)
    eyer = np.zeros((1, 256), np.float32)
    oh = np.zeros((16, 2048), np.float32)
    for h in range(NH):
        eyer[0, 16 * h + h] = 1.0
        oh[h, 128 * h:128 * (h + 1)] = 1.0

    # master[pj, plane, u]: r = u - pj - 2048 (= i_local - j_local)
    u = np.arange(MW, dtype=np.float64)[None, :]
    pj = np.arange(128, dtype=np.float64)[:, None]
    r = u - pj - 2048.0
    p0 = -np.abs(r)
    def _bf(a):
        return np.ascontiguousarray(
            np.maximum(a, -20000.0).astype(np.float32)
        ).astype(ml_dtypes.bfloat16)

    m0 = _bf(p0)
    masters1 = [_bf(-np.abs(r[:, :2048])), _bf(-np.abs(r[:, :2048] + 2048.0))]

    in_maps = []
    for c in range(8):
        b, t = c // 2, c % 2
        xr = np.ascontiguousarray(np.roll(x[b], -QTOK * t, axis=0))
        in_maps.append({
            "x": xr, "wq": wq_a, "wk": wk_a, "wv": wv_a, "wo": wo_a,
            "master": m0, "master1": masters1[t], "ones": ones, "ident": ident, "eyerow": eyer, "oh16": oh,
        })
    return s_heads, Ts, in_maps


def kernel(**inputs):
    global LAST_EXEC_NS
    s_heads, Ts, in_maps = _prep(**inputs)
    nc = _build_graph(s_heads, Ts)
    trace = os.environ.get("KERNEL_TRACE") == "1"
    res = run_bass_kernel_spmd(
        nc, in_maps, core_ids=list(range(8)), trace=trace
    )
    LAST_EXEC_NS = res.exec_time_ns
    out = np.empty((4, CTX, DIM), np.float32)
    for c in range(8):
        b, t = c // 2, c % 2
        out[b, QTOK * t:QTOK * (t + 1), :] = res.results[c]["out"]
    return out


# revision 34
# speedup vs baseline: 1.0165x; 1.0165x over previous
"""Distributed Bass kernel: LN + multi-head ALiBi attention + out-proj.

Sharding: 8 cores = (batch b in 0..3) x (query-token half t in 0..1).
Each core computes the full pipeline for its 1024 query tokens (all 16
heads); K/V are computed for the full 2048-token sequence (duplicated
across the 2 cores of a batch).  No collectives.

SPMD trick: every core runs the SAME graph.  Core (b, t) receives x[b]
rolled by -1024*t along tokens, so its query tokens always sit at local
rows 0..1023.  The ALiBi distance table ("master") is per-core DATA
encoding true global distances (two planes: j-tiles < 8 and >= 8, which
for t=1 differ by a 2048 wrap).  Blocks a core computes needlessly are
killed by the bias (exp(-large) ~ 0).

ALiBi banding: head h only effectively attends within |i-j| <= T_h =
ceil(23/s_h) (dropped softmax mass < ~1e-7 of the denominator), so
score blocks outside the band are skipped statically.

The per-head 1/(8*s_h) is folded into Wq on the host; the ALiBi bias is
added on DVE (PSUM + master slice), and ACT exp applies scale=s_h, so
exp(s_h*(QK/(8 s_h) + master)) is the softmax numerator.  The softmax
denominator arrives as a ones-column of the PV matmul; per q-chunk all
16 heads' sums are gathered into one PSUM tile via one-hot matmuls so a
SINGLE Ln serves the chunk (avoids ACT table-set thrashing), then a
per-head ones-matmul broadcast + exp(-x) forms 1/l across partitions
and OT is normalized in place.
"""

import os
import sys

sys.path.insert(0, "/opt/trn_rl_repo")

import numpy as np
import ml_dtypes

import concourse.bass as bass
import concourse.mybir as mybir
import concourse.tile as tile
from concourse import bacc
from concourse.bass import ts
from concourse.bass_utils import run_bass_kernel_spmd

BF16 = mybir.dt.bfloat16
F32 = mybir.dt.float32
F32R = mybir.dt.float32r

CTX = 2048
DIM = 1024
NH = 16
DH = 64
QTOK = 1024  # query tokens per core
EPS = 1e-5
MW = 3072  # master table width

LAST_EXEC_NS = None


def _band_blocks(T, qc):
    """j-tile list for query chunk qc (local g0 = qc*512), band half-width T."""
    g0 = qc * 512
    lo = max(0, g0 - T) // 128
    hi = (min(CTX, g0 + 512 + T) + 127) // 128
    jts = set(range(lo, hi))
    if qc == 0 and T < CTX:
        # wrap blocks: j_local in [2048-T, 2048) carries the left band of the
        # t=1 core (j_global ~ 1024-T..1024); bias-killed garbage for t=0.
        jts |= set(range((CTX - T) // 128, CTX // 128))
    return sorted(jts)


def _build_graph(s_heads, Ts):
    """Build the shared SPMD Bass graph; returns compiled nc."""
    nc = bacc.Bacc("TRN2", target_bir_lowering=False, debug=False)

    x_d = nc.dram_tensor("x", [CTX, DIM], F32, kind="ExternalInput").ap()
    wq_d = nc.dram_tensor("wq", [8, 128, 8, 128], BF16, kind="ExternalInput").ap()
    wk_d = nc.dram_tensor("wk", [8, 128, 8, 128], BF16, kind="ExternalInput").ap()
    wv_d = nc.dram_tensor("wv", [8, 128, DIM], BF16, kind="ExternalInput").ap()
    wo_d = nc.dram_tensor("wo", [8, 128, DIM], BF16, kind="ExternalInput").ap()
    mst_d = nc.dram_tensor("master", [128, MW], BF16, kind="ExternalInput").ap()
    mst1_d = nc.dram_tensor("master1", [128, 2048], BF16, kind="ExternalInput").ap()
    one_d = nc.dram_tensor("ones", [1, 128], F32R, kind="ExternalInput").ap()
    idn_d = nc.dram_tensor("ident", [128, 128], BF16, kind="ExternalInput").ap()
    eye_d = nc.dram_tensor("eyerow", [1, 256], F32R, kind="ExternalInput").ap()
    oh_d = nc.dram_tensor("oh16", [16, 2048], F32R, kind="ExternalInput").ap()
    out_d = nc.dram_tensor("out", [QTOK, DIM], F32, kind="ExternalOutput").ap()

    AF = mybir.ActivationFunctionType
    ALU = mybir.AluOpType

    with tile.TileContext(nc) as tc:
        with (
            tc.tile_pool(name="persist", bufs=1) as pp,
            tc.tile_pool(name="dram", bufs=1, space="DRAM") as dp,
            tc.tile_pool(name="xio", bufs=2) as xp,
            tc.tile_pool(name="xnp", bufs=4) as xnp,
            tc.tile_pool(name="wstream", bufs=3) as wp,
            tc.tile_pool(name="ptile", bufs=2) as ptp,
            tc.tile_pool(name="small", bufs=4) as sp,
            tc.tile_pool(name="small2", bufs=2) as sp2,
            tc.tile_pool(name="norm", bufs=2) as epool,
            tc.tile_pool(name="wopool", bufs=1) as wop,
            tc.tile_pool(name="opool", bufs=2) as op,
            tc.tile_pool(name="ps_proj", bufs=2, space="PSUM") as ps_proj,
            tc.tile_pool(name="ps_s", bufs=2, space="PSUM") as ps_s,
            tc.tile_pool(name="ps_o", bufs=2, space="PSUM") as ps_o,
            tc.tile_pool(name="ps_n", bufs=1, space="PSUM") as ps_n,
        ):
            # ---- persistent SBUF ----
            master = pp.tile([128, MW], BF16, tag="master")
            master1 = pp.tile([128, 2048], BF16, tag="master1")
            ones = pp.tile([1, 128], F32R, tag="ones")
            ident = pp.tile([128, 128], BF16, tag="ident")
            eyer = pp.tile([1, 256], F32R, tag="eyer")
            oh16 = pp.tile([16, 2048], F32R, tag="oh16")
            xnT = pp.tile([128, 8, CTX], BF16, tag="big")  # slot shared with OT
            KT = pp.tile([128, 8, CTX], BF16, tag="KT")
            QT = pp.tile([128, 8, QTOK], BF16, tag="QT")
            Vsb = pp.tile([128, 16, NH, 65], BF16, tag="Vsb")
            wv_sb = pp.tile([128, 8, DIM], BF16, tag="wv")

            nc.sync.dma_start(master[:], mst_d[:])
            nc.sync.dma_start(master1[:], mst1_d[:])
            nc.sync.dma_start(ones[:], one_d[:])
            nc.sync.dma_start(ident[:], idn_d[:])
            nc.sync.dma_start(eyer[:], eye_d[:])
            nc.sync.dma_start(oh16[:], oh_d[:])
            eps_sb = pp.tile([128, 1], F32, tag="eps")
            nc.any.memset(eps_sb[:], EPS)
            nc.any.memset(Vsb[:, :, :, 64:65], 1.0)
            nc.sync.dma_start(wv_sb[:], wv_d.rearrange("k p d -> p k d"))

            # ---- Phases A+B+C pipelined per 512-token chunk ----
            for ch in range(4):
                mv4 = sp.tile([128, 4, 2], F32, tag="mv4")
                for i, tt in enumerate(range(4 * ch, 4 * ch + 4)):
                    xt = xp.tile([128, DIM], F32, tag="xt")
                    nc.sync.dma_start(xt[:], x_d[ts(tt, 128), :])
                    st6 = sp.tile([128, 2, 6], F32, tag="st6")
                    nc.vector.bn_stats(st6[:, 0, :], xt[:, 0:512])
                    nc.vector.bn_stats(st6[:, 1, :], xt[:, 512:1024])
                    nc.vector.bn_aggr(mv4[:, i, :], st6[:])
                # rstd for 4 tiles in one Ln + one Exp (keeps table set put)
                lv4 = sp.tile([128, 4], F32, tag="lv4")
                nc.scalar.activation(lv4[:], mv4[:, :, 1], AF.Ln, bias=eps_sb[:])
                rs4 = sp.tile([128, 4], F32, tag="rs4")
                nc.scalar.activation(rs4[:], lv4[:], AF.Exp, scale=-0.5)
                xns = []
                for i, tt in enumerate(range(4 * ch, 4 * ch + 4)):
                    xt = xp.tile([128, DIM], F32, tag="xt")
                    nc.sync.dma_start(xt[:], x_d[ts(tt, 128), :])
                    xn = xnp.tile([128, DIM], BF16, tag="xn")
                    nc.vector.tensor_scalar(
                        xn[:], xt[:], mv4[:, i, 0:1], rs4[:, i:i + 1],
                        ALU.subtract, ALU.mult,
                    )
                    xns.append(xn)
                for ko in range(8):  # transpose on PE (proj PSUM pool reused)
                    tp = ps_proj.tile([128, 512], F32, tag="pp")
                    tpv = tp[:].bitcast(BF16)
                    for i in range(4):
                        nc.tensor.transpose(
                            tpv[:, ts(i, 128)], xns[i][:, ts(ko, 128)], ident[:]
                        )
                    nc.any.tensor_copy(xnT[:, ko, ts(ch, 512)], tpv[:, 0:512])
                for dqt in range(8):  # K^T chunk
                    wkt = wp.tile([128, 8, 128], BF16, tag="wqt")
                    nc.scalar.dma_start(wkt[:], wk_d[dqt])
                    ps = ps_proj.tile([128, 512], F32, tag="pp")
                    for ko in range(8):
                        nc.tensor.matmul(
                            ps[:], wkt[:, ko, :], xnT[:, ko, ts(ch, 512)],
                            start=(ko == 0), stop=(ko == 7),
                        )
                    nc.vector.tensor_copy(KT[:, dqt, ts(ch, 512)], ps[:])
                if ch < 2:
                    for dqt in range(8):  # Q^T chunk
                        wqt = wp.tile([128, 8, 128], BF16, tag="wqt")
                        nc.scalar.dma_start(wqt[:], wq_d[dqt])
                        ps = ps_proj.tile([128, 512], F32, tag="pp")
                        for ko in range(8):
                            nc.tensor.matmul(
                                ps[:], wqt[:, ko, :], xnT[:, ko, ts(ch, 512)],
                                start=(ko == 0), stop=(ko == 7),
                            )
                        nc.vector.tensor_copy(QT[:, dqt, ts(ch, 512)], ps[:])
                for jt in range(4 * ch, 4 * ch + 4):  # V chunk
                    for dvc in range(2):
                        ps = ps_proj.tile([128, 512], F32, tag="pp")
                        for ko in range(8):
                            nc.tensor.matmul(
                                ps[:], xnT[:, ko, ts(jt, 128)],
                                wv_sb[:, ko, ts(dvc, 512)],
                                start=(ko == 0), stop=(ko == 7),
                            )
                        nc.vector.tensor_copy(
                            Vsb[:, jt, dvc * 8:(dvc + 1) * 8, 0:64],
                            ps[:].rearrange("p (h d) -> p h d", d=64),
                        )

            # OT shares the xnT slot (all xnT readers are above)
            OT = pp.tile([128, 8, QTOK], BF16, tag="big")

            # ---- Phase D: banded attention (qc outer; batched softmax norm) --
            # Heads are processed in pairs occupying PE row-groups [0:64] and
            # [64:128]; their score matmuls run concurrently on the array.
            head_order = list(range(NH))
            head_groups = [(h,) for h in head_order]
            for qc in range(QTOK // 512):
                lcol = ps_n.tile([16, 512], F32, tag="lcol")
                for grp_heads in head_groups:
                    blocks = _band_blocks(max(Ts[h] for h in grp_heads), qc)
                    nb = len(blocks)
                    po = {}
                    for h in grp_heads:
                        po[h] = ps_o.tile([65, 512], F32, tag="po", name=f"po{h}")
                    for b0 in range(0, nb, 2):
                        grp = blocks[b0:b0 + 2]
                        g = len(grp)
                        ein = {}
                        pt = {}
                        for h in grp_heads:
                            ein[h] = ptp.tile([128, 1024], F32, tag="ein", name=f"ein{h}")
                            pt[h] = ptp.tile([128, 1024], BF16, tag="pt", name=f"pt{h}")
                        for gi, jt in enumerate(grp):
                            pss = {}
                            for h in grp_heads:
                                base = 64 * (h % 2)
                                dqt = h // 2
                                pss[h] = ps_s.tile([128, 512], F32, tag="ps", name=f"pss{h}")
                                nc.tensor.matmul(
                                    pss[h][:],
                                    KT[base:base + 64, dqt, ts(jt, 128)],
                                    QT[base:base + 64, dqt, ts(qc, 512)],
                                    start=True, stop=True,
                                )
                            off = 2048 + 512 * qc - 128 * jt
                            msrc = master1 if jt >= 8 else master
                            for h in grp_heads:
                                nc.vector.tensor_tensor(
                                    ein[h][:, ts(gi, 512)], pss[h][:],
                                    msrc[:, off:off + 512], ALU.add,
                                )
                        for h in grp_heads:
                            nc.scalar.activation(
                                pt[h][:, 0:g * 512], ein[h][:, 0:g * 512],
                                AF.Exp, scale=float(s_heads[h]),
                            )
                        for gi, jt in enumerate(grp):
                            for h in grp_heads:
                                nc.tensor.matmul(
                                    po[h][:], Vsb[:, jt, h, :],
                                    pt[h][:, ts(gi, 512)],
                                    start=(b0 + gi == 0),
                                    stop=(b0 + gi == nb - 1),
                                )
                    for h in grp_heads:
                        base = 64 * (h % 2)
                        dqt = h // 2
                        # gather this head's softmax sums into the shared lcol
                        lt = sp2.tile([1, 512], F32R, tag="ltmp")
                        nc.scalar.copy(lt[:], po[h][64:65, :])
                        nc.tensor.matmul(
                            lcol[:], eyer[0:1, ts(h, 16)], lt[:],
                            start=(h == head_order[0]),
                            stop=(h == head_order[-1]),
                        )
                        # stash unnormalized O^T
                        if base == 0:
                            nc.any.tensor_copy(
                                OT[0:64, dqt, ts(qc, 512)], po[h][0:64, :]
                            )
                        else:
                            tmp = epool.tile([64, 512], BF16, tag="otmp")
                            nc.any.tensor_copy(tmp[:], po[h][0:64, :])
                            nc.sync.dma_start(
                                OT[64:128, dqt, ts(qc, 512)], tmp[:]
                            )
                # batched softmax normalization: one Ln for all 16 heads
                lnl = sp2.tile([16, 512], F32R, tag="lnl16")
                nc.scalar.activation(lnl[:], lcol[:], AF.Ln)
                for h in range(NH):
                    base = 64 * (h % 2)
                    dqt = h // 2
                    pb = ps_n.tile([128, 512], F32, tag="pb")
                    nc.tensor.matmul(
                        pb[:], oh16[:, ts(h, 128)], lnl[:],
                        start=True, stop=True,
                    )
                    einv = epool.tile([128, 512], BF16, tag="einv")
                    nc.scalar.activation(einv[:], pb[:], AF.Exp, scale=-1.0)
                    nc.vector.tensor_tensor(
                        OT[base:base + 64, dqt, ts(qc, 512)],
                        OT[base:base + 64, dqt, ts(qc, 512)],
                        einv[base:base + 64, :], ALU.mult,
                    )
                # ---- output projection for this q-half (overlaps next qc) ----
                for ec in range(2):
                    wot = wop.tile([128, 8, 512], BF16, tag="wot")
                    nc.sync.dma_start(
                        wot[:], wo_d[:, :, ts(ec, 512)].rearrange("h p e -> p h e")
                    )
                    for it in range(4 * qc, 4 * qc + 4):
                        ps = ps_proj.tile([128, 512], F32, tag="pp")
                        for hdt in range(8):
                            nc.tensor.matmul(
                                ps[:], OT[:, hdt, ts(it, 128)], wot[:, hdt, :],
                                start=(hdt == 0), stop=(hdt == 7),
                            )
                        ot = op.tile([128, 512], F32, tag="ot")
                        nc.any.tensor_copy(ot[:], ps[:])
                        nc.sync.dma_start(out_d[ts(it, 128), ts(ec, 512)], ot[:])

    nc.compile()
    return nc


def _prep(x, ln_w, ln_b, Wq, Wk, Wv, Wo, M):
    """Host-side input preparation -> (s_heads, Ts, in_maps)."""
    x = np.asarray(x, np.float32)
    ln_w = np.asarray(ln_w, np.float32)
    ln_b = np.asarray(ln_b, np.float32)
    Wq = np.asarray(Wq, np.float32)
    Wk = np.asarray(Wk, np.float32)
    Wv = np.asarray(Wv, np.float32)
    Wo = np.asarray(Wo, np.float32)
    M = np.asarray(M, np.float32)
    assert not np.any(ln_b), "kernel assumes ln_b == 0 (folded LN bias unsupported)"

    s_heads = (-M[:, 0, 1]).astype(np.float64)  # M[h,0,1] = -s_h
    Ts = [min(CTX, int(np.ceil(23.0 / s))) for s in s_heads]

    wq_eff = ln_w[:, None] * Wq
    for h in range(NH):
        wq_eff[:, h * DH:(h + 1) * DH] /= 8.0 * s_heads[h]
    wk_eff = ln_w[:, None] * Wk
    wv_eff = ln_w[:, None] * Wv

    def wq_layout(w):  # [1024,1024] -> [dqt, p, ko, m]
        return np.ascontiguousarray(
            w.reshape(8, 128, 8, 128).transpose(2, 1, 0, 3)
        ).astype(ml_dtypes.bfloat16)

    wq_a = wq_layout(wq_eff)
    wk_a = wq_layout(wk_eff)
    wv_a = np.ascontiguousarray(wv_eff.reshape(8, 128, DIM)).astype(
        ml_dtypes.bfloat16
    )
    wo_a = np.ascontiguousarray(Wo.reshape(8, 128, DIM)).astype(ml_dtypes.bfloat16)

    ones = np.ones((1, 128), np.float32)
    ident = np.eye(128, dtype=np.float32).astype(ml_dtypes.bfloat16)
    eyer = np.zeros((1, 256), np.float32)
    oh = np.zeros((16, 2048), np.float32)
    for h in range(NH):
        eyer[0, 16 * h + h] = 1.0
        oh[h, 128 * h:128 * (h + 1)] = 1.0

    # master[pj, plane, u]: r = u - pj - 2048 (= i_local - j_local)
    u = np.arange(MW, dtype=np.float64)[None, :]
    pj = np.arange(128, dtype=np.float64)[:, None]
    r = u - pj - 2048.0
    p0 = -np.abs(r)
    def _bf(a):
        return np.ascontiguousarray(
            np.maximum(a, -20000.0).astype(np.float32)
        ).astype(ml_dtypes.bfloat16)

    m0 = _bf(p0)
    masters1 = [_bf(-np.abs(r[:, :2048])), _bf(-np.abs(r[:, :2048] + 2048.0))]

    in_maps = []
    for c in range(8):
        b, t = c // 2, c % 2
        xr = np.ascontiguousarray(np.roll(x[b], -QTOK * t, axis=0))
        in_maps.append({
            "x": xr, "wq": wq_a, "wk": wk_a, "wv": wv_a, "wo": wo_a,
            "master": m0, "master1": masters1[t], "ones": ones, "ident": ident, "eyerow": eyer, "oh16": oh,
        })
    return s_heads, Ts, in_maps


def kernel(**inputs):
    global LAST_EXEC_NS
    s_heads, Ts, in_maps = _prep(**inputs)
    nc = _build_graph(s_heads, Ts)
    trace = os.environ.get("KERNEL_TRACE") == "1"
    res = run_bass_kernel_spmd(
        nc, in_maps, core_ids=list(range(8)), trace=trace
    )
    LAST_EXEC_NS = res.exec_time_ns
    out = np.empty((4, CTX, DIM), np.float32)
    for c in range(8):
        b, t = c // 2, c % 2
        out[b, QTOK * t:QTOK * (t + 1), :] = res.results[c]["out"]
    return out


# revision 35
# speedup vs baseline: 1.0987x; 1.0809x over previous
"""Distributed Bass kernel: LN + multi-head ALiBi attention + out-proj.

Sharding: 8 cores = (batch b in 0..3) x (query-token half t in 0..1).
Each core computes the full pipeline for its 1024 query tokens (all 16
heads); K/V are computed for the full 2048-token sequence (duplicated
across the 2 cores of a batch).  No collectives.

SPMD trick: every core runs the SAME graph.  Core (b, t) receives x[b]
rolled by -1024*t along tokens, so its query tokens always sit at local
rows 0..1023.  The ALiBi distance table ("master") is per-core DATA
encoding true global distances (two planes: j-tiles < 8 and >= 8, which
for t=1 differ by a 2048 wrap).  Blocks a core computes needlessly are
killed by the bias (exp(-large) ~ 0).

ALiBi banding: head h only effectively attends within |i-j| <= T_h =
ceil(23/s_h) (dropped softmax mass < ~1e-7 of the denominator), so
score blocks outside the band are skipped statically.

The per-head 1/(8*s_h) is folded into Wq on the host; the ALiBi bias is
added on DVE (PSUM + master slice), and ACT exp applies scale=s_h, so
exp(s_h*(QK/(8 s_h) + master)) is the softmax numerator.  The softmax
denominator arrives as a ones-column of the PV matmul; per q-chunk all
16 heads' sums are gathered into one PSUM tile via one-hot matmuls so a
SINGLE Ln serves the chunk (avoids ACT table-set thrashing), then a
per-head ones-matmul broadcast + exp(-x) forms 1/l across partitions
and OT is normalized in place.
"""

import os
import sys

sys.path.insert(0, "/opt/trn_rl_repo")

import numpy as np
import ml_dtypes

import concourse.bass as bass
import concourse.mybir as mybir
import concourse.tile as tile
from concourse import bacc
from concourse.bass import ts
from concourse.bass_utils import run_bass_kernel_spmd

BF16 = mybir.dt.bfloat16
F32 = mybir.dt.float32
F32R = mybir.dt.float32r

CTX = 2048
DIM = 1024
NH = 16
DH = 64
QTOK = 1024  # query tokens per core
EPS = 1e-5
MW = 3072  # master table width

LAST_EXEC_NS = None


def _band_blocks(T, qc):
    """j-tile list for query chunk qc (local g0 = qc*512), band half-width T."""
    g0 = qc * 512
    lo = max(0, g0 - T) // 128
    hi = (min(CTX, g0 + 512 + T) + 127) // 128
    jts = set(range(lo, hi))
    if qc == 0 and T < CTX:
        # wrap blocks: j_local in [2048-T, 2048) carries the left band of the
        # t=1 core (j_global ~ 1024-T..1024); bias-killed garbage for t=0.
        jts |= set(range((CTX - T) // 128, CTX // 128))
    return sorted(jts)


def _build_graph(s_heads, Ts):
    """Build the shared SPMD Bass graph; returns compiled nc."""
    nc = bacc.Bacc("TRN2", target_bir_lowering=False, debug=False)

    x_d = nc.dram_tensor("x", [CTX, DIM], F32, kind="ExternalInput").ap()
    wq_d = nc.dram_tensor("wq", [8, 128, 8, 128], BF16, kind="ExternalInput").ap()
    wk_d = nc.dram_tensor("wk", [8, 128, 8, 128], BF16, kind="ExternalInput").ap()
    wv_d = nc.dram_tensor("wv", [8, 128, DIM], BF16, kind="ExternalInput").ap()
    wo_d = nc.dram_tensor("wo", [8, 128, DIM], BF16, kind="ExternalInput").ap()
    mst_d = nc.dram_tensor("master", [128, MW], BF16, kind="ExternalInput").ap()
    mst1_d = nc.dram_tensor("master1", [128, 2048], BF16, kind="ExternalInput").ap()
    one_d = nc.dram_tensor("ones", [1, 128], F32R, kind="ExternalInput").ap()
    idn_d = nc.dram_tensor("ident", [128, 128], BF16, kind="ExternalInput").ap()
    eye_d = nc.dram_tensor("eyerow", [1, 256], F32R, kind="ExternalInput").ap()
    oh_d = nc.dram_tensor("oh16", [16, 2048], F32R, kind="ExternalInput").ap()
    out_d = nc.dram_tensor("out", [QTOK, DIM], F32, kind="ExternalOutput").ap()

    AF = mybir.ActivationFunctionType
    ALU = mybir.AluOpType

    with tile.TileContext(nc) as tc:
        with (
            tc.tile_pool(name="persist", bufs=1) as pp,
            tc.tile_pool(name="dram", bufs=1, space="DRAM") as dp,
            tc.tile_pool(name="xio", bufs=2) as xp,
            tc.tile_pool(name="xnp", bufs=4) as xnp,
            tc.tile_pool(name="wstream", bufs=3) as wp,
            tc.tile_pool(name="ptile", bufs=2) as ptp,
            tc.tile_pool(name="small", bufs=4) as sp,
            tc.tile_pool(name="small2", bufs=2) as sp2,
            tc.tile_pool(name="norm", bufs=2) as epool,
            tc.tile_pool(name="wopool", bufs=1) as wop,
            tc.tile_pool(name="opool", bufs=2) as op,
            tc.tile_pool(name="ps_proj", bufs=2, space="PSUM") as ps_proj,
            tc.tile_pool(name="ps_s", bufs=3, space="PSUM") as ps_s,
            tc.tile_pool(name="ps_o", bufs=2, space="PSUM") as ps_o,
            tc.tile_pool(name="ps_n", bufs=1, space="PSUM") as ps_n,
        ):
            # ---- persistent SBUF ----
            master = pp.tile([128, MW], BF16, tag="master")
            master1 = pp.tile([128, 2048], BF16, tag="master1")
            ones = pp.tile([1, 128], F32R, tag="ones")
            ident = pp.tile([128, 128], BF16, tag="ident")
            eyer = pp.tile([1, 256], F32R, tag="eyer")
            oh16 = pp.tile([16, 2048], F32R, tag="oh16")
            xnT = pp.tile([128, 8, CTX], BF16, tag="big")  # slot shared with OT
            KT = pp.tile([128, 8, CTX], BF16, tag="KT")
            QT = pp.tile([128, 8, QTOK], BF16, tag="QT")
            Vsb = pp.tile([128, 16, NH, 65], BF16, tag="Vsb")
            wv_sb = pp.tile([128, 8, DIM], BF16, tag="wv")

            nc.sync.dma_start(master[:], mst_d[:])
            nc.sync.dma_start(master1[:], mst1_d[:])
            nc.sync.dma_start(ones[:], one_d[:])
            nc.sync.dma_start(ident[:], idn_d[:])
            nc.sync.dma_start(eyer[:], eye_d[:])
            nc.sync.dma_start(oh16[:], oh_d[:])
            eps_sb = pp.tile([128, 1], F32, tag="eps")
            nc.any.memset(eps_sb[:], EPS)
            nc.any.memset(Vsb[:, :, :, 64:65], 1.0)
            nc.sync.dma_start(wv_sb[:], wv_d.rearrange("k p d -> p k d"))

            # ---- Phases A+B+C pipelined per 512-token chunk ----
            for ch in range(4):
                mv4 = sp.tile([128, 4, 2], F32, tag="mv4")
                for i, tt in enumerate(range(4 * ch, 4 * ch + 4)):
                    xt = xp.tile([128, DIM], F32, tag="xt")
                    nc.sync.dma_start(xt[:], x_d[ts(tt, 128), :])
                    st6 = sp.tile([128, 2, 6], F32, tag="st6")
                    nc.vector.bn_stats(st6[:, 0, :], xt[:, 0:512])
                    nc.vector.bn_stats(st6[:, 1, :], xt[:, 512:1024])
                    nc.vector.bn_aggr(mv4[:, i, :], st6[:])
                # rstd for 4 tiles in one Ln + one Exp (keeps table set put)
                lv4 = sp.tile([128, 4], F32, tag="lv4")
                nc.scalar.activation(lv4[:], mv4[:, :, 1], AF.Ln, bias=eps_sb[:])
                rs4 = sp.tile([128, 4], F32, tag="rs4")
                nc.scalar.activation(rs4[:], lv4[:], AF.Exp, scale=-0.5)
                xns = []
                for i, tt in enumerate(range(4 * ch, 4 * ch + 4)):
                    xt = xp.tile([128, DIM], F32, tag="xt")
                    nc.sync.dma_start(xt[:], x_d[ts(tt, 128), :])
                    xn = xnp.tile([128, DIM], BF16, tag="xn")
                    nc.vector.tensor_scalar(
                        xn[:], xt[:], mv4[:, i, 0:1], rs4[:, i:i + 1],
                        ALU.subtract, ALU.mult,
                    )
                    xns.append(xn)
                for ko in range(8):  # transpose on PE (proj PSUM pool reused)
                    tp = ps_proj.tile([128, 512], F32, tag="pp")
                    tpv = tp[:].bitcast(BF16)
                    for i in range(4):
                        nc.tensor.transpose(
                            tpv[:, ts(i, 128)], xns[i][:, ts(ko, 128)], ident[:]
                        )
                    nc.any.tensor_copy(xnT[:, ko, ts(ch, 512)], tpv[:, 0:512])
                for dqt in range(8):  # K^T chunk
                    wkt = wp.tile([128, 8, 128], BF16, tag="wqt")
                    nc.scalar.dma_start(wkt[:], wk_d[dqt])
                    ps = ps_proj.tile([128, 512], F32, tag="pp")
                    for ko in range(8):
                        nc.tensor.matmul(
                            ps[:], wkt[:, ko, :], xnT[:, ko, ts(ch, 512)],
                            start=(ko == 0), stop=(ko == 7),
                        )
                    nc.vector.tensor_copy(KT[:, dqt, ts(ch, 512)], ps[:])
                if ch < 2:
                    for dqt in range(8):  # Q^T chunk
                        wqt = wp.tile([128, 8, 128], BF16, tag="wqt")
                        nc.scalar.dma_start(wqt[:], wq_d[dqt])
                        ps = ps_proj.tile([128, 512], F32, tag="pp")
                        for ko in range(8):
                            nc.tensor.matmul(
                                ps[:], wqt[:, ko, :], xnT[:, ko, ts(ch, 512)],
                                start=(ko == 0), stop=(ko == 7),
                            )
                        nc.vector.tensor_copy(QT[:, dqt, ts(ch, 512)], ps[:])
                for jt in range(4 * ch, 4 * ch + 4):  # V chunk
                    for dvc in range(2):
                        ps = ps_proj.tile([128, 512], F32, tag="pp")
                        for ko in range(8):
                            nc.tensor.matmul(
                                ps[:], xnT[:, ko, ts(jt, 128)],
                                wv_sb[:, ko, ts(dvc, 512)],
                                start=(ko == 0), stop=(ko == 7),
                            )
                        nc.vector.tensor_copy(
                            Vsb[:, jt, dvc * 8:(dvc + 1) * 8, 0:64],
                            ps[:].rearrange("p (h d) -> p h d", d=64),
                        )

            # OT shares the xnT slot (all xnT readers are above)
            OT = pp.tile([128, 8, QTOK], BF16, tag="big")

            # ---- Phase D: banded attention (qc outer; batched softmax norm) --
            # Heads are processed in pairs occupying PE row-groups [0:64] and
            # [64:128]; their score matmuls run concurrently on the array.
            head_order = list(range(NH))
            head_groups = [(h,) for h in head_order]
            for qc in range(QTOK // 512):
                lcol = ps_n.tile([16, 512], F32, tag="lcol")
                for grp_heads in head_groups:
                    blocks = _band_blocks(max(Ts[h] for h in grp_heads), qc)
                    nb = len(blocks)
                    po = {}
                    for h in grp_heads:
                        po[h] = ps_o.tile([65, 512], F32, tag="po", name=f"po{h}")
                    for b0 in range(0, nb, 2):
                        grp = blocks[b0:b0 + 2]
                        g = len(grp)
                        ein = {}
                        pt = {}
                        for h in grp_heads:
                            ein[h] = ptp.tile([128, 1024], F32, tag="ein", name=f"ein{h}")
                            pt[h] = ptp.tile([128, 1024], BF16, tag="pt", name=f"pt{h}")
                        for gi, jt in enumerate(grp):
                            pss = {}
                            for h in grp_heads:
                                base = 64 * (h % 2)
                                dqt = h // 2
                                pss[h] = ps_s.tile([128, 512], F32, tag="ps", name=f"pss{h}")
                                nc.tensor.matmul(
                                    pss[h][:],
                                    KT[base:base + 64, dqt, ts(jt, 128)],
                                    QT[base:base + 64, dqt, ts(qc, 512)],
                                    start=True, stop=True,
                                )
                            off = 2048 + 512 * qc - 128 * jt
                            msrc = master1 if jt >= 8 else master
                            for h in grp_heads:
                                nc.vector.tensor_tensor(
                                    ein[h][:, ts(gi, 512)], pss[h][:],
                                    msrc[:, off:off + 512], ALU.add,
                                )
                        for h in grp_heads:
                            nc.scalar.activation(
                                pt[h][:, 0:g * 512], ein[h][:, 0:g * 512],
                                AF.Exp, scale=float(s_heads[h]),
                            )
                        for gi, jt in enumerate(grp):
                            for h in grp_heads:
                                nc.tensor.matmul(
                                    po[h][:], Vsb[:, jt, h, :],
                                    pt[h][:, ts(gi, 512)],
                                    start=(b0 + gi == 0),
                                    stop=(b0 + gi == nb - 1),
                                )
                    for h in grp_heads:
                        base = 64 * (h % 2)
                        dqt = h // 2
                        # gather this head's softmax sums into the shared lcol
                        lt = sp2.tile([1, 512], F32R, tag="ltmp")
                        nc.scalar.copy(lt[:], po[h][64:65, :])
                        nc.tensor.matmul(
                            lcol[:], eyer[0:1, ts(h, 16)], lt[:],
                            start=(h == head_order[0]),
                            stop=(h == head_order[-1]),
                        )
                        # stash unnormalized O^T
                        if base == 0:
                            nc.any.tensor_copy(
                                OT[0:64, dqt, ts(qc, 512)], po[h][0:64, :]
                            )
                        else:
                            tmp = epool.tile([64, 512], BF16, tag="otmp")
                            nc.any.tensor_copy(tmp[:], po[h][0:64, :])
                            nc.sync.dma_start(
                                OT[64:128, dqt, ts(qc, 512)], tmp[:]
                            )
                # batched softmax normalization: one Ln for all 16 heads
                lnl = sp2.tile([16, 512], F32R, tag="lnl16")
                nc.scalar.activation(lnl[:], lcol[:], AF.Ln)
                for h in range(NH):
                    base = 64 * (h % 2)
                    dqt = h // 2
                    pb = ps_proj.tile([128, 512], F32, tag="pp", name=f"pb{h}")
                    nc.tensor.matmul(
                        pb[:], oh16[:, ts(h, 128)], lnl[:],
                        start=True, stop=True,
                    )
                    einv = epool.tile([128, 512], BF16, tag="einv")
                    nc.scalar.activation(einv[:], pb[:], AF.Exp, scale=-1.0)
                    nc.vector.tensor_tensor(
                        OT[base:base + 64, dqt, ts(qc, 512)],
                        OT[base:base + 64, dqt, ts(qc, 512)],
                        einv[base:base + 64, :], ALU.mult,
                    )
                # ---- output projection for this q-half (overlaps next qc) ----
                for ec in range(2):
                    wot = wop.tile([128, 8, 512], BF16, tag="wot")
                    nc.sync.dma_start(
                        wot[:], wo_d[:, :, ts(ec, 512)].rearrange("h p e -> p h e")
                    )
                    for it in range(4 * qc, 4 * qc + 4):
                        ps = ps_proj.tile([128, 512], F32, tag="pp")
                        for hdt in range(8):
                            nc.tensor.matmul(
                                ps[:], OT[:, hdt, ts(it, 128)], wot[:, hdt, :],
                                start=(hdt == 0), stop=(hdt == 7),
                            )
                        ot = op.tile([128, 512], F32, tag="ot")
                        nc.any.tensor_copy(ot[:], ps[:])
                        nc.sync.dma_start(out_d[ts(it, 128), ts(ec, 512)], ot[:])

    nc.compile()
    return nc


def _prep(x, ln_w, ln_b, Wq, Wk, Wv, Wo, M):
    """Host-side input preparation -> (s_heads, Ts, in_maps)."""
    x = np.asarray(x, np.float32)
    ln_w = np.asarray(ln_w, np.float32)
    ln_b = np.asarray(ln_b, np.float32)
    Wq = np.asarray(Wq, np.float32)
    Wk = np.asarray(Wk, np.float32)
    Wv = np.asarray(Wv, np.float32)
    Wo = np.asarray(Wo, np.float32)
    M = np.asarray(M, np.float32)
    assert not np.any(ln_b), "kernel assumes ln_b == 0 (folded LN bias unsupported)"

    s_heads = (-M[:, 0, 1]).astype(np.float64)  # M[h,0,1] = -s_h
    Ts = [min(CTX, int(np.ceil(23.0 / s))) for s in s_heads]

    wq_eff = ln_w[:, None] * Wq
    for h in range(NH):
        wq_eff[:, h * DH:(h + 1) * DH] /= 8.0 * s_heads[h]
    wk_eff = ln_w[:, None] * Wk
    wv_eff = ln_w[:, None] * Wv

    def wq_layout(w):  # [1024,1024] -> [dqt, p, ko, m]
        return np.ascontiguousarray(
            w.reshape(8, 128, 8, 128).transpose(2, 1, 0, 3)
        ).astype(ml_dtypes.bfloat16)

    wq_a = wq_layout(wq_eff)
    wk_a = wq_layout(wk_eff)
    wv_a = np.ascontiguousarray(wv_eff.reshape(8, 128, DIM)).astype(
        ml_dtypes.bfloat16
    )
    wo_a = np.ascontiguousarray(Wo.reshape(8, 128, DIM)).astype(ml_dtypes.bfloat16)

    ones = np.ones((1, 128), np.float32)
    ident = np.eye(128, dtype=np.float32).astype(ml_dtypes.bfloat16)
    eyer = np.zeros((1, 256), np.float32)
    oh = np.zeros((16, 2048), np.float32)
    for h in range(NH):
        eyer[0, 16 * h + h] = 1.0
        oh[h, 128 * h:128 * (h + 1)] = 1.0

    # master[pj, plane, u]: r = u - pj - 2048 (= i_local - j_local)
    u = np.arange(MW, dtype=np.float64)[None, :]
    pj = np.arange(128, dtype=np.float64)[:, None]
    r = u - pj - 2048.0
    p0 = -np.abs(r)
    def _bf(a):
        return np.ascontiguousarray(
            np.maximum(a, -20000.0).astype(np.float32)
        ).astype(ml_dtypes.bfloat16)

    m0 = _bf(p0)
    masters1 = [_bf(-np.abs(r[:, :2048])), _bf(-np.abs(r[:, :2048] + 2048.0))]

    in_maps = []
    for c in range(8):
        b, t = c // 2, c % 2
        xr = np.ascontiguousarray(np.roll(x[b], -QTOK * t, axis=0))
        in_maps.append({
            "x": xr, "wq": wq_a, "wk": wk_a, "wv": wv_a, "wo": wo_a,
            "master": m0, "master1": masters1[t], "ones": ones, "ident": ident, "eyerow": eyer, "oh16": oh,
        })
    return s_heads, Ts, in_maps


def kernel(**inputs):
    global LAST_EXEC_NS
    s_heads, Ts, in_maps = _prep(**inputs)
    nc = _build_graph(s_heads, Ts)
    trace = os.environ.get("KERNEL_TRACE") == "1"
    res = run_bass_kernel_spmd(
        nc, in_maps, core_ids=list(range(8)), trace=trace
    )
    LAST_EXEC_NS = res.exec_time_ns
    out = np.empty((4, CTX, DIM), np.float32)
    for c in range(8):
        b, t = c // 2, c % 2
        out[b, QTOK * t:QTOK * (t + 1), :] = res.results[c]["out"]
    return out


# revision 36
# speedup vs baseline: 1.1311x; 1.0294x over previous
"""Distributed Bass kernel: LN + multi-head ALiBi attention + out-proj.

Sharding: 8 cores = (batch b in 0..3) x (query-token half t in 0..1).
Each core computes the full pipeline for its 1024 query tokens (all 16
heads); K/V are computed for the full 2048-token sequence (duplicated
across the 2 cores of a batch).  No collectives.

SPMD trick: every core runs the SAME graph.  Core (b, t) receives x[b]
rolled by -1024*t along tokens, so its query tokens always sit at local
rows 0..1023.  The ALiBi distance table ("master") is per-core DATA
encoding true global distances (two planes: j-tiles < 8 and >= 8, which
for t=1 differ by a 2048 wrap).  Blocks a core computes needlessly are
killed by the bias (exp(-large) ~ 0).

ALiBi banding: head h only effectively attends within |i-j| <= T_h =
ceil(23/s_h) (dropped softmax mass < ~1e-7 of the denominator), so
score blocks outside the band are skipped statically.

The per-head 1/(8*s_h) is folded into Wq on the host; the ALiBi bias is
added on DVE (PSUM + master slice), and ACT exp applies scale=s_h, so
exp(s_h*(QK/(8 s_h) + master)) is the softmax numerator.  The softmax
denominator arrives as a ones-column of the PV matmul; per q-chunk all
16 heads' sums are gathered into one PSUM tile via one-hot matmuls so a
SINGLE Ln serves the chunk (avoids ACT table-set thrashing), then a
per-head ones-matmul broadcast + exp(-x) forms 1/l across partitions
and OT is normalized in place.
"""

import os
import sys

sys.path.insert(0, "/opt/trn_rl_repo")

import numpy as np
import ml_dtypes

import concourse.bass as bass
import concourse.mybir as mybir
import concourse.tile as tile
from concourse import bacc
from concourse.bass import ts
from concourse.bass_utils import run_bass_kernel_spmd

BF16 = mybir.dt.bfloat16
F32 = mybir.dt.float32
F32R = mybir.dt.float32r

CTX = 2048
DIM = 1024
NH = 16
DH = 64
QTOK = 1024  # query tokens per core
EPS = 1e-5
MW = 3072  # master table width

LAST_EXEC_NS = None


def _band_blocks(T, qc):
    """j-tile list for query chunk qc (local g0 = qc*512), band half-width T."""
    g0 = qc * 512
    lo = max(0, g0 - T) // 128
    hi = (min(CTX, g0 + 512 + T) + 127) // 128
    jts = set(range(lo, hi))
    if qc == 0 and T < CTX:
        # wrap blocks: j_local in [2048-T, 2048) carries the left band of the
        # t=1 core (j_global ~ 1024-T..1024); bias-killed garbage for t=0.
        jts |= set(range((CTX - T) // 128, CTX // 128))
    return sorted(jts)


def _build_graph(s_heads, Ts):
    """Build the shared SPMD Bass graph; returns compiled nc."""
    nc = bacc.Bacc("TRN2", target_bir_lowering=False, debug=False)

    x_d = nc.dram_tensor("x", [CTX, DIM], F32, kind="ExternalInput").ap()
    wq_d = nc.dram_tensor("wq", [8, 128, 8, 128], BF16, kind="ExternalInput").ap()
    wk_d = nc.dram_tensor("wk", [8, 128, 8, 128], BF16, kind="ExternalInput").ap()
    wv_d = nc.dram_tensor("wv", [8, 128, DIM], BF16, kind="ExternalInput").ap()
    wo_d = nc.dram_tensor("wo", [8, 128, DIM], BF16, kind="ExternalInput").ap()
    mst_d = nc.dram_tensor("master", [128, MW], BF16, kind="ExternalInput").ap()
    mst1_d = nc.dram_tensor("master1", [128, 2048], BF16, kind="ExternalInput").ap()
    one_d = nc.dram_tensor("ones", [1, 128], F32R, kind="ExternalInput").ap()
    idn_d = nc.dram_tensor("ident", [128, 128], BF16, kind="ExternalInput").ap()
    eye_d = nc.dram_tensor("eyerow", [1, 256], F32R, kind="ExternalInput").ap()
    oh_d = nc.dram_tensor("oh16", [16, 2048], F32R, kind="ExternalInput").ap()
    out_d = nc.dram_tensor("out", [QTOK, DIM], F32, kind="ExternalOutput").ap()

    AF = mybir.ActivationFunctionType
    ALU = mybir.AluOpType

    with tile.TileContext(nc) as tc:
        with (
            tc.tile_pool(name="persist", bufs=1) as pp,
            tc.tile_pool(name="dram", bufs=1, space="DRAM") as dp,
            tc.tile_pool(name="xio", bufs=2) as xp,
            tc.tile_pool(name="xnp", bufs=4) as xnp,
            tc.tile_pool(name="wstream", bufs=3) as wp,
            tc.tile_pool(name="ptile", bufs=2) as ptp,
            tc.tile_pool(name="small", bufs=4) as sp,
            tc.tile_pool(name="small2", bufs=2) as sp2,
            tc.tile_pool(name="norm", bufs=2) as epool,
            tc.tile_pool(name="wopool", bufs=1) as wop,
            tc.tile_pool(name="opool", bufs=2) as op,
            tc.tile_pool(name="ps_proj", bufs=2, space="PSUM") as ps_proj,
            tc.tile_pool(name="ps_s", bufs=3, space="PSUM") as ps_s,
            tc.tile_pool(name="ps_o", bufs=2, space="PSUM") as ps_o,
            tc.tile_pool(name="ps_n", bufs=1, space="PSUM") as ps_n,
        ):
            # ---- persistent SBUF ----
            master = pp.tile([128, MW], BF16, tag="master")
            master1 = pp.tile([128, 2048], BF16, tag="master1")
            ones = pp.tile([1, 128], F32R, tag="ones")
            ident = pp.tile([128, 128], BF16, tag="ident")
            eyer = pp.tile([1, 256], F32R, tag="eyer")
            oh16 = pp.tile([16, 2048], F32R, tag="oh16")
            xnT = pp.tile([128, 8, CTX], BF16, tag="big")  # slot shared with OT
            KT = pp.tile([128, 8, CTX], BF16, tag="KT")
            QT = pp.tile([128, 8, QTOK], BF16, tag="QT")
            Vsb = pp.tile([128, 16, NH, 65], BF16, tag="Vsb")
            wv_sb = pp.tile([128, 8, DIM], BF16, tag="wv")

            nc.scalar.dma_start(master[:], mst_d[:])
            nc.scalar.dma_start(master1[:], mst1_d[:])
            nc.scalar.dma_start(ones[:], one_d[:])
            nc.scalar.dma_start(ident[:], idn_d[:])
            nc.scalar.dma_start(eyer[:], eye_d[:])
            nc.scalar.dma_start(oh16[:], oh_d[:])
            eps_sb = pp.tile([128, 1], F32, tag="eps")
            nc.any.memset(eps_sb[:], EPS)
            nc.any.memset(Vsb[:, :, :, 64:65], 1.0)
            nc.scalar.dma_start(wv_sb[:], wv_d.rearrange("k p d -> p k d"))

            # ---- Phases A+B+C pipelined per 512-token chunk ----
            for ch in range(4):
                mv4 = sp.tile([128, 4, 2], F32, tag="mv4")
                for i, tt in enumerate(range(4 * ch, 4 * ch + 4)):
                    xt = xp.tile([128, DIM], F32, tag="xt")
                    nc.sync.dma_start(xt[:], x_d[ts(tt, 128), :])
                    st6 = sp.tile([128, 2, 6], F32, tag="st6")
                    nc.vector.bn_stats(st6[:, 0, :], xt[:, 0:512])
                    nc.vector.bn_stats(st6[:, 1, :], xt[:, 512:1024])
                    nc.vector.bn_aggr(mv4[:, i, :], st6[:])
                # rstd for 4 tiles in one Ln + one Exp (keeps table set put)
                lv4 = sp.tile([128, 4], F32, tag="lv4")
                nc.scalar.activation(lv4[:], mv4[:, :, 1], AF.Ln, bias=eps_sb[:])
                rs4 = sp.tile([128, 4], F32, tag="rs4")
                nc.scalar.activation(rs4[:], lv4[:], AF.Exp, scale=-0.5)
                xns = []
                for i, tt in enumerate(range(4 * ch, 4 * ch + 4)):
                    xt = xp.tile([128, DIM], F32, tag="xt")
                    nc.sync.dma_start(xt[:], x_d[ts(tt, 128), :])
                    xn = xnp.tile([128, DIM], BF16, tag="xn")
                    nc.vector.tensor_scalar(
                        xn[:], xt[:], mv4[:, i, 0:1], rs4[:, i:i + 1],
                        ALU.subtract, ALU.mult,
                    )
                    xns.append(xn)
                for ko in range(8):  # transpose on PE (proj PSUM pool reused)
                    tp = ps_proj.tile([128, 512], F32, tag="pp")
                    tpv = tp[:].bitcast(BF16)
                    for i in range(4):
                        nc.tensor.transpose(
                            tpv[:, ts(i, 128)], xns[i][:, ts(ko, 128)], ident[:]
                        )
                    nc.any.tensor_copy(xnT[:, ko, ts(ch, 512)], tpv[:, 0:512])
                for dqt in range(8):  # K^T chunk
                    wkt = wp.tile([128, 8, 128], BF16, tag="wqt")
                    nc.scalar.dma_start(wkt[:], wk_d[dqt])
                    ps = ps_proj.tile([128, 512], F32, tag="pp")
                    for ko in range(8):
                        nc.tensor.matmul(
                            ps[:], wkt[:, ko, :], xnT[:, ko, ts(ch, 512)],
                            start=(ko == 0), stop=(ko == 7),
                        )
                    nc.vector.tensor_copy(KT[:, dqt, ts(ch, 512)], ps[:])
                if ch < 2:
                    for dqt in range(8):  # Q^T chunk
                        wqt = wp.tile([128, 8, 128], BF16, tag="wqt")
                        nc.scalar.dma_start(wqt[:], wq_d[dqt])
                        ps = ps_proj.tile([128, 512], F32, tag="pp")
                        for ko in range(8):
                            nc.tensor.matmul(
                                ps[:], wqt[:, ko, :], xnT[:, ko, ts(ch, 512)],
                                start=(ko == 0), stop=(ko == 7),
                            )
                        nc.vector.tensor_copy(QT[:, dqt, ts(ch, 512)], ps[:])
                for jt in range(4 * ch, 4 * ch + 4):  # V chunk
                    for dvc in range(2):
                        ps = ps_proj.tile([128, 512], F32, tag="pp")
                        for ko in range(8):
                            nc.tensor.matmul(
                                ps[:], xnT[:, ko, ts(jt, 128)],
                                wv_sb[:, ko, ts(dvc, 512)],
                                start=(ko == 0), stop=(ko == 7),
                            )
                        nc.vector.tensor_copy(
                            Vsb[:, jt, dvc * 8:(dvc + 1) * 8, 0:64],
                            ps[:].rearrange("p (h d) -> p h d", d=64),
                        )

            # OT shares the xnT slot (all xnT readers are above)
            OT = pp.tile([128, 8, QTOK], BF16, tag="big")

            # ---- Phase D: banded attention (qc outer; batched softmax norm) --
            # Heads are processed in pairs occupying PE row-groups [0:64] and
            # [64:128]; their score matmuls run concurrently on the array.
            head_order = list(range(NH))
            head_groups = [(h,) for h in head_order]
            for qc in range(QTOK // 512):
                lcol = ps_n.tile([16, 512], F32, tag="lcol")
                for grp_heads in head_groups:
                    blocks = _band_blocks(max(Ts[h] for h in grp_heads), qc)
                    nb = len(blocks)
                    po = {}
                    for h in grp_heads:
                        po[h] = ps_o.tile([65, 512], F32, tag="po", name=f"po{h}")
                    for b0 in range(0, nb, 2):
                        grp = blocks[b0:b0 + 2]
                        g = len(grp)
                        ein = {}
                        pt = {}
                        for h in grp_heads:
                            ein[h] = ptp.tile([128, 1024], F32, tag="ein", name=f"ein{h}")
                            pt[h] = ptp.tile([128, 1024], BF16, tag="pt", name=f"pt{h}")
                        for gi, jt in enumerate(grp):
                            pss = {}
                            for h in grp_heads:
                                base = 64 * (h % 2)
                                dqt = h // 2
                                pss[h] = ps_s.tile([128, 512], F32, tag="ps", name=f"pss{h}")
                                nc.tensor.matmul(
                                    pss[h][:],
                                    KT[base:base + 64, dqt, ts(jt, 128)],
                                    QT[base:base + 64, dqt, ts(qc, 512)],
                                    start=True, stop=True,
                                )
                            off = 2048 + 512 * qc - 128 * jt
                            msrc = master1 if jt >= 8 else master
                            for h in grp_heads:
                                nc.vector.tensor_tensor(
                                    ein[h][:, ts(gi, 512)], pss[h][:],
                                    msrc[:, off:off + 512], ALU.add,
                                )
                        for h in grp_heads:
                            nc.scalar.activation(
                                pt[h][:, 0:g * 512], ein[h][:, 0:g * 512],
                                AF.Exp, scale=float(s_heads[h]),
                            )
                        for gi, jt in enumerate(grp):
                            for h in grp_heads:
                                nc.tensor.matmul(
                                    po[h][:], Vsb[:, jt, h, :],
                                    pt[h][:, ts(gi, 512)],
                                    start=(b0 + gi == 0),
                                    stop=(b0 + gi == nb - 1),
                                )
                    for h in grp_heads:
                        base = 64 * (h % 2)
                        dqt = h // 2
                        # gather this head's softmax sums into the shared lcol
                        lt = sp2.tile([1, 512], F32R, tag="ltmp")
                        nc.scalar.copy(lt[:], po[h][64:65, :])
                        nc.tensor.matmul(
                            lcol[:], eyer[0:1, ts(h, 16)], lt[:],
                            start=(h == head_order[0]),
                            stop=(h == head_order[-1]),
                        )
                        # stash unnormalized O^T
                        if base == 0:
                            nc.any.tensor_copy(
                                OT[0:64, dqt, ts(qc, 512)], po[h][0:64, :]
                            )
                        else:
                            tmp = epool.tile([64, 512], BF16, tag="otmp")
                            nc.any.tensor_copy(tmp[:], po[h][0:64, :])
                            nc.sync.dma_start(
                                OT[64:128, dqt, ts(qc, 512)], tmp[:]
                            )
                # batched softmax normalization: one Ln for all 16 heads
                lnl = sp2.tile([16, 512], F32R, tag="lnl16")
                nc.scalar.activation(lnl[:], lcol[:], AF.Ln)
                for h in range(NH):
                    base = 64 * (h % 2)
                    dqt = h // 2
                    pb = ps_proj.tile([128, 512], F32, tag="pp", name=f"pb{h}")
                    nc.tensor.matmul(
                        pb[:], oh16[:, ts(h, 128)], lnl[:],
                        start=True, stop=True,
                    )
                    einv = epool.tile([128, 512], BF16, tag="einv")
                    nc.scalar.activation(einv[:], pb[:], AF.Exp, scale=-1.0)
                    nc.vector.tensor_tensor(
                        OT[base:base + 64, dqt, ts(qc, 512)],
                        OT[base:base + 64, dqt, ts(qc, 512)],
                        einv[base:base + 64, :], ALU.mult,
                    )
                # ---- output projection for this q-half (overlaps next qc) ----
                for ec in range(2):
                    wot = wop.tile([128, 8, 512], BF16, tag="wot")
                    nc.sync.dma_start(
                        wot[:], wo_d[:, :, ts(ec, 512)].rearrange("h p e -> p h e")
                    )
                    for it in range(4 * qc, 4 * qc + 4):
                        ps = ps_proj.tile([128, 512], F32, tag="pp")
                        for hdt in range(8):
                            nc.tensor.matmul(
                                ps[:], OT[:, hdt, ts(it, 128)], wot[:, hdt, :],
                                start=(hdt == 0), stop=(hdt == 7),
                            )
                        ot = op.tile([128, 512], F32, tag="ot")
                        nc.any.tensor_copy(ot[:], ps[:])
                        nc.sync.dma_start(out_d[ts(it, 128), ts(ec, 512)], ot[:])

    nc.compile()
    return nc


def _prep(x, ln_w, ln_b, Wq, Wk, Wv, Wo, M):
    """Host-side input preparation -> (s_heads, Ts, in_maps)."""
    x = np.asarray(x, np.float32)
    ln_w = np.asarray(ln_w, np.float32)
    ln_b = np.asarray(ln_b, np.float32)
    Wq = np.asarray(Wq, np.float32)
    Wk = np.asarray(Wk, np.float32)
    Wv = np.asarray(Wv, np.float32)
    Wo = np.asarray(Wo, np.float32)
    M = np.asarray(M, np.float32)
    assert not np.any(ln_b), "kernel assumes ln_b == 0 (folded LN bias unsupported)"

    s_heads = (-M[:, 0, 1]).astype(np.float64)  # M[h,0,1] = -s_h
    Ts = [min(CTX, int(np.ceil(23.0 / s))) for s in s_heads]

    wq_eff = ln_w[:, None] * Wq
    for h in range(NH):
        wq_eff[:, h * DH:(h + 1) * DH] /= 8.0 * s_heads[h]
    wk_eff = ln_w[:, None] * Wk
    wv_eff = ln_w[:, None] * Wv

    def wq_layout(w):  # [1024,1024] -> [dqt, p, ko, m]
        return np.ascontiguousarray(
            w.reshape(8, 128, 8, 128).transpose(2, 1, 0, 3)
        ).astype(ml_dtypes.bfloat16)

    wq_a = wq_layout(wq_eff)
    wk_a = wq_layout(wk_eff)
    wv_a = np.ascontiguousarray(wv_eff.reshape(8, 128, DIM)).astype(
        ml_dtypes.bfloat16
    )
    wo_a = np.ascontiguousarray(Wo.reshape(8, 128, DIM)).astype(ml_dtypes.bfloat16)

    ones = np.ones((1, 128), np.float32)
    ident = np.eye(128, dtype=np.float32).astype(ml_dtypes.bfloat16)
    eyer = np.zeros((1, 256), np.float32)
    oh = np.zeros((16, 2048), np.float32)
    for h in range(NH):
        eyer[0, 16 * h + h] = 1.0
        oh[h, 128 * h:128 * (h + 1)] = 1.0

    # master[pj, plane, u]: r = u - pj - 2048 (= i_local - j_local)
    u = np.arange(MW, dtype=np.float64)[None, :]
    pj = np.arange(128, dtype=np.float64)[:, None]
    r = u - pj - 2048.0
    p0 = -np.abs(r)
    def _bf(a):
        return np.ascontiguousarray(
            np.maximum(a, -20000.0).astype(np.float32)
        ).astype(ml_dtypes.bfloat16)

    m0 = _bf(p0)
    masters1 = [_bf(-np.abs(r[:, :2048])), _bf(-np.abs(r[:, :2048] + 2048.0))]

    in_maps = []
    for c in range(8):
        b, t = c // 2, c % 2
        xr = np.ascontiguousarray(np.roll(x[b], -QTOK * t, axis=0))
        in_maps.append({
            "x": xr, "wq": wq_a, "wk": wk_a, "wv": wv_a, "wo": wo_a,
            "master": m0, "master1": masters1[t], "ones": ones, "ident": ident, "eyerow": eyer, "oh16": oh,
        })
    return s_heads, Ts, in_maps


def kernel(**inputs):
    global LAST_EXEC_NS
    s_heads, Ts, in_maps = _prep(**inputs)
    nc = _build_graph(s_heads, Ts)
    trace = os.environ.get("KERNEL_TRACE") == "1"
    res = run_bass_kernel_spmd(
        nc, in_maps, core_ids=list(range(8)), trace=trace
    )
    LAST_EXEC_NS = res.exec_time_ns
    out = np.empty((4, CTX, DIM), np.float32)
    for c in range(8):
        b, t = c // 2, c % 2
        out[b, QTOK * t:QTOK * (t + 1), :] = res.results[c]["out"]
    return out


# revision 37
# speedup vs baseline: 1.1782x; 1.0417x over previous
"""Distributed Bass kernel: LN + multi-head ALiBi attention + out-proj.

Sharding: 8 cores = (batch b in 0..3) x (query-token half t in 0..1).
Each core computes the full pipeline for its 1024 query tokens (all 16
heads); K/V are computed for the full 2048-token sequence (duplicated
across the 2 cores of a batch).  No collectives.

SPMD trick: every core runs the SAME graph.  Core (b, t) receives x[b]
rolled by -1024*t along tokens, so its query tokens always sit at local
rows 0..1023.  The ALiBi distance table ("master") is per-core DATA
encoding true global distances (two planes: j-tiles < 8 and >= 8, which
for t=1 differ by a 2048 wrap).  Blocks a core computes needlessly are
killed by the bias (exp(-large) ~ 0).

ALiBi banding: head h only effectively attends within |i-j| <= T_h =
ceil(23/s_h) (dropped softmax mass < ~1e-7 of the denominator), so
score blocks outside the band are skipped statically.

The per-head 1/(8*s_h) is folded into Wq on the host; the ALiBi bias is
added on DVE (PSUM + master slice), and ACT exp applies scale=s_h, so
exp(s_h*(QK/(8 s_h) + master)) is the softmax numerator.  The softmax
denominator arrives as a ones-column of the PV matmul; per q-chunk all
16 heads' sums are gathered into one PSUM tile via one-hot matmuls so a
SINGLE Ln serves the chunk (avoids ACT table-set thrashing), then a
per-head ones-matmul broadcast + exp(-x) forms 1/l across partitions
and OT is normalized in place.
"""

import os
import sys

sys.path.insert(0, "/opt/trn_rl_repo")

import numpy as np
import ml_dtypes

import concourse.bass as bass
import concourse.mybir as mybir
import concourse.tile as tile
from concourse import bacc
from concourse.bass import ts
from concourse.bass_utils import run_bass_kernel_spmd

BF16 = mybir.dt.bfloat16
F32 = mybir.dt.float32
F32R = mybir.dt.float32r

CTX = 2048
DIM = 1024
NH = 16
DH = 64
QTOK = 1024  # query tokens per core
EPS = 1e-5
MW = 3072  # master table width

LAST_EXEC_NS = None


def _band_blocks(T, qc):
    """j-tile list for query chunk qc (local g0 = qc*512), band half-width T."""
    g0 = qc * 512
    lo = max(0, g0 - T) // 128
    hi = (min(CTX, g0 + 512 + T) + 127) // 128
    jts = set(range(lo, hi))
    if qc == 0 and T < CTX:
        # wrap blocks: j_local in [2048-T, 2048) carries the left band of the
        # t=1 core (j_global ~ 1024-T..1024); bias-killed garbage for t=0.
        jts |= set(range((CTX - T) // 128, CTX // 128))
    return sorted(jts)


def _build_graph(s_heads, Ts):
    """Build the shared SPMD Bass graph; returns compiled nc."""
    nc = bacc.Bacc("TRN2", target_bir_lowering=False, debug=False)

    x_d = nc.dram_tensor("x", [CTX, DIM], BF16, kind="ExternalInput").ap()
    wq_d = nc.dram_tensor("wq", [8, 128, 8, 128], BF16, kind="ExternalInput").ap()
    wk_d = nc.dram_tensor("wk", [8, 128, 8, 128], BF16, kind="ExternalInput").ap()
    wv_d = nc.dram_tensor("wv", [8, 128, DIM], BF16, kind="ExternalInput").ap()
    wo_d = nc.dram_tensor("wo", [8, 128, DIM], BF16, kind="ExternalInput").ap()
    mst_d = nc.dram_tensor("master", [128, MW], BF16, kind="ExternalInput").ap()
    mst1_d = nc.dram_tensor("master1", [128, 2048], BF16, kind="ExternalInput").ap()
    one_d = nc.dram_tensor("ones", [1, 128], F32R, kind="ExternalInput").ap()
    idn_d = nc.dram_tensor("ident", [128, 128], BF16, kind="ExternalInput").ap()
    eye_d = nc.dram_tensor("eyerow", [1, 256], F32R, kind="ExternalInput").ap()
    oh_d = nc.dram_tensor("oh16", [16, 2048], F32R, kind="ExternalInput").ap()
    out_d = nc.dram_tensor("out", [QTOK, DIM], F32, kind="ExternalOutput").ap()

    AF = mybir.ActivationFunctionType
    ALU = mybir.AluOpType

    with tile.TileContext(nc) as tc:
        with (
            tc.tile_pool(name="persist", bufs=1) as pp,
            tc.tile_pool(name="dram", bufs=1, space="DRAM") as dp,
            tc.tile_pool(name="xio", bufs=2) as xp,
            tc.tile_pool(name="xnp", bufs=4) as xnp,
            tc.tile_pool(name="wstream", bufs=3) as wp,
            tc.tile_pool(name="ptile", bufs=2) as ptp,
            tc.tile_pool(name="small", bufs=4) as sp,
            tc.tile_pool(name="small2", bufs=2) as sp2,
            tc.tile_pool(name="norm", bufs=2) as epool,
            tc.tile_pool(name="wopool", bufs=1) as wop,
            tc.tile_pool(name="opool", bufs=2) as op,
            tc.tile_pool(name="ps_proj", bufs=2, space="PSUM") as ps_proj,
            tc.tile_pool(name="ps_s", bufs=3, space="PSUM") as ps_s,
            tc.tile_pool(name="ps_o", bufs=2, space="PSUM") as ps_o,
            tc.tile_pool(name="ps_n", bufs=1, space="PSUM") as ps_n,
        ):
            # ---- persistent SBUF ----
            master = pp.tile([128, MW], BF16, tag="master")
            master1 = pp.tile([128, 2048], BF16, tag="master1")
            ones = pp.tile([1, 128], F32R, tag="ones")
            ident = pp.tile([128, 128], BF16, tag="ident")
            eyer = pp.tile([1, 256], F32R, tag="eyer")
            oh16 = pp.tile([16, 2048], F32R, tag="oh16")
            xnT = pp.tile([128, 8, CTX], BF16, tag="big")  # slot shared with OT
            KT = pp.tile([128, 8, CTX], BF16, tag="KT")
            QT = pp.tile([128, 8, QTOK], BF16, tag="QT")
            Vsb = pp.tile([128, 16, NH, 65], BF16, tag="Vsb")
            wv_sb = pp.tile([128, 8, DIM], BF16, tag="wv")

            nc.scalar.dma_start(master[:], mst_d[:])
            nc.scalar.dma_start(master1[:], mst1_d[:])
            nc.scalar.dma_start(ones[:], one_d[:])
            nc.scalar.dma_start(ident[:], idn_d[:])
            nc.scalar.dma_start(eyer[:], eye_d[:])
            nc.scalar.dma_start(oh16[:], oh_d[:])
            eps_sb = pp.tile([128, 1], F32, tag="eps")
            nc.any.memset(eps_sb[:], EPS)
            nc.any.memset(Vsb[:, :, :, 64:65], 1.0)
            nc.scalar.dma_start(wv_sb[:], wv_d.rearrange("k p d -> p k d"))

            # ---- Phases A+B+C pipelined per 512-token chunk ----
            for ch in range(4):
                mv4 = sp.tile([128, 4, 2], F32, tag="mv4")
                for i, tt in enumerate(range(4 * ch, 4 * ch + 4)):
                    xt = xp.tile([128, DIM], BF16, tag="xt")
                    nc.sync.dma_start(xt[:], x_d[ts(tt, 128), :])
                    st6 = sp.tile([128, 2, 6], F32, tag="st6")
                    nc.vector.bn_stats(st6[:, 0, :], xt[:, 0:512])
                    nc.vector.bn_stats(st6[:, 1, :], xt[:, 512:1024])
                    nc.vector.bn_aggr(mv4[:, i, :], st6[:])
                # rstd for 4 tiles in one Ln + one Exp (keeps table set put)
                lv4 = sp.tile([128, 4], F32, tag="lv4")
                nc.scalar.activation(lv4[:], mv4[:, :, 1], AF.Ln, bias=eps_sb[:])
                rs4 = sp.tile([128, 4], F32, tag="rs4")
                nc.scalar.activation(rs4[:], lv4[:], AF.Exp, scale=-0.5)
                xns = []
                for i, tt in enumerate(range(4 * ch, 4 * ch + 4)):
                    xt = xp.tile([128, DIM], BF16, tag="xt")
                    nc.sync.dma_start(xt[:], x_d[ts(tt, 128), :])
                    xn = xnp.tile([128, DIM], BF16, tag="xn")
                    nc.vector.tensor_scalar(
                        xn[:], xt[:], mv4[:, i, 0:1], rs4[:, i:i + 1],
                        ALU.subtract, ALU.mult,
                    )
                    xns.append(xn)
                for ko in range(8):  # transpose on PE (proj PSUM pool reused)
                    tp = ps_proj.tile([128, 512], F32, tag="pp")
                    tpv = tp[:].bitcast(BF16)
                    for i in range(4):
                        nc.tensor.transpose(
                            tpv[:, ts(i, 128)], xns[i][:, ts(ko, 128)], ident[:]
                        )
                    nc.any.tensor_copy(xnT[:, ko, ts(ch, 512)], tpv[:, 0:512])
                for dqt in range(8):  # K^T chunk
                    wkt = wp.tile([128, 8, 128], BF16, tag="wqt")
                    nc.scalar.dma_start(wkt[:], wk_d[dqt])
                    ps = ps_proj.tile([128, 512], F32, tag="pp")
                    for ko in range(8):
                        nc.tensor.matmul(
                            ps[:], wkt[:, ko, :], xnT[:, ko, ts(ch, 512)],
                            start=(ko == 0), stop=(ko == 7),
                        )
                    nc.vector.tensor_copy(KT[:, dqt, ts(ch, 512)], ps[:])
                if ch < 2:
                    for dqt in range(8):  # Q^T chunk
                        wqt = wp.tile([128, 8, 128], BF16, tag="wqt")
                        nc.scalar.dma_start(wqt[:], wq_d[dqt])
                        ps = ps_proj.tile([128, 512], F32, tag="pp")
                        for ko in range(8):
                            nc.tensor.matmul(
                                ps[:], wqt[:, ko, :], xnT[:, ko, ts(ch, 512)],
                                start=(ko == 0), stop=(ko == 7),
                            )
                        nc.vector.tensor_copy(QT[:, dqt, ts(ch, 512)], ps[:])
                for jt in range(4 * ch, 4 * ch + 4):  # V chunk
                    for dvc in range(2):
                        ps = ps_proj.tile([128, 512], F32, tag="pp")
                        for ko in range(8):
                            nc.tensor.matmul(
                                ps[:], xnT[:, ko, ts(jt, 128)],
                                wv_sb[:, ko, ts(dvc, 512)],
                                start=(ko == 0), stop=(ko == 7),
                            )
                        nc.vector.tensor_copy(
                            Vsb[:, jt, dvc * 8:(dvc + 1) * 8, 0:64],
                            ps[:].rearrange("p (h d) -> p h d", d=64),
                        )

            # OT shares the xnT slot (all xnT readers are above)
            OT = pp.tile([128, 8, QTOK], BF16, tag="big")

            # ---- Phase D: banded attention (qc outer; batched softmax norm) --
            # Heads are processed in pairs occupying PE row-groups [0:64] and
            # [64:128]; their score matmuls run concurrently on the array.
            head_order = list(range(NH))
            head_groups = [(h,) for h in head_order]
            for qc in range(QTOK // 512):
                lcol = ps_n.tile([16, 512], F32, tag="lcol")
                for grp_heads in head_groups:
                    blocks = _band_blocks(max(Ts[h] for h in grp_heads), qc)
                    nb = len(blocks)
                    po = {}
                    for h in grp_heads:
                        po[h] = ps_o.tile([65, 512], F32, tag="po", name=f"po{h}")
                    for b0 in range(0, nb, 2):
                        grp = blocks[b0:b0 + 2]
                        g = len(grp)
                        ein = {}
                        pt = {}
                        for h in grp_heads:
                            ein[h] = ptp.tile([128, 1024], F32, tag="ein", name=f"ein{h}")
                            pt[h] = ptp.tile([128, 1024], BF16, tag="pt", name=f"pt{h}")
                        for gi, jt in enumerate(grp):
                            pss = {}
                            for h in grp_heads:
                                base = 64 * (h % 2)
                                dqt = h // 2
                                pss[h] = ps_s.tile([128, 512], F32, tag="ps", name=f"pss{h}")
                                nc.tensor.matmul(
                                    pss[h][:],
                                    KT[base:base + 64, dqt, ts(jt, 128)],
                                    QT[base:base + 64, dqt, ts(qc, 512)],
                                    start=True, stop=True,
                                )
                            off = 2048 + 512 * qc - 128 * jt
                            msrc = master1 if jt >= 8 else master
                            for h in grp_heads:
                                nc.vector.tensor_tensor(
                                    ein[h][:, ts(gi, 512)], pss[h][:],
                                    msrc[:, off:off + 512], ALU.add,
                                )
                        for h in grp_heads:
                            nc.scalar.activation(
                                pt[h][:, 0:g * 512], ein[h][:, 0:g * 512],
                                AF.Exp, scale=float(s_heads[h]),
                            )
                        for gi, jt in enumerate(grp):
                            for h in grp_heads:
                                nc.tensor.matmul(
                                    po[h][:], Vsb[:, jt, h, :],
                                    pt[h][:, ts(gi, 512)],
                                    start=(b0 + gi == 0),
                                    stop=(b0 + gi == nb - 1),
                                )
                    for h in grp_heads:
                        base = 64 * (h % 2)
                        dqt = h // 2
                        # gather this head's softmax sums into the shared lcol
                        lt = sp2.tile([1, 512], F32R, tag="ltmp")
                        nc.scalar.copy(lt[:], po[h][64:65, :])
                        nc.tensor.matmul(
                            lcol[:], eyer[0:1, ts(h, 16)], lt[:],
                            start=(h == head_order[0]),
                            stop=(h == head_order[-1]),
                        )
                        # stash unnormalized O^T
                        if base == 0:
                            nc.any.tensor_copy(
                                OT[0:64, dqt, ts(qc, 512)], po[h][0:64, :]
                            )
                        else:
                            tmp = epool.tile([64, 512], BF16, tag="otmp")
                            nc.any.tensor_copy(tmp[:], po[h][0:64, :])
                            nc.sync.dma_start(
                                OT[64:128, dqt, ts(qc, 512)], tmp[:]
                            )
                # batched softmax normalization: one Ln for all 16 heads
                lnl = sp2.tile([16, 512], F32R, tag="lnl16")
                nc.scalar.activation(lnl[:], lcol[:], AF.Ln)
                for h in range(NH):
                    base = 64 * (h % 2)
                    dqt = h // 2
                    pb = ps_proj.tile([128, 512], F32, tag="pp", name=f"pb{h}")
                    nc.tensor.matmul(
                        pb[:], oh16[:, ts(h, 128)], lnl[:],
                        start=True, stop=True,
                    )
                    einv = epool.tile([128, 512], BF16, tag="einv")
                    nc.scalar.activation(einv[:], pb[:], AF.Exp, scale=-1.0)
                    nc.vector.tensor_tensor(
                        OT[base:base + 64, dqt, ts(qc, 512)],
                        OT[base:base + 64, dqt, ts(qc, 512)],
                        einv[base:base + 64, :], ALU.mult,
                    )
                # ---- output projection for this q-half (overlaps next qc) ----
                for ec in range(2):
                    wot = wop.tile([128, 8, 512], BF16, tag="wot")
                    nc.sync.dma_start(
                        wot[:], wo_d[:, :, ts(ec, 512)].rearrange("h p e -> p h e")
                    )
                    for it in range(4 * qc, 4 * qc + 4):
                        ps = ps_proj.tile([128, 512], F32, tag="pp")
                        for hdt in range(8):
                            nc.tensor.matmul(
                                ps[:], OT[:, hdt, ts(it, 128)], wot[:, hdt, :],
                                start=(hdt == 0), stop=(hdt == 7),
                            )
                        ot = op.tile([128, 512], F32, tag="ot")
                        nc.any.tensor_copy(ot[:], ps[:])
                        nc.sync.dma_start(out_d[ts(it, 128), ts(ec, 512)], ot[:])

    nc.compile()
    return nc


def _prep(x, ln_w, ln_b, Wq, Wk, Wv, Wo, M):
    """Host-side input preparation -> (s_heads, Ts, in_maps)."""
    x = np.asarray(x, np.float32)
    ln_w = np.asarray(ln_w, np.float32)
    ln_b = np.asarray(ln_b, np.float32)
    Wq = np.asarray(Wq, np.float32)
    Wk = np.asarray(Wk, np.float32)
    Wv = np.asarray(Wv, np.float32)
    Wo = np.asarray(Wo, np.float32)
    M = np.asarray(M, np.float32)
    assert not np.any(ln_b), "kernel assumes ln_b == 0 (folded LN bias unsupported)"

    s_heads = (-M[:, 0, 1]).astype(np.float64)  # M[h,0,1] = -s_h
    Ts = [min(CTX, int(np.ceil(21.0 / s))) for s in s_heads]

    wq_eff = ln_w[:, None] * Wq
    for h in range(NH):
        wq_eff[:, h * DH:(h + 1) * DH] /= 8.0 * s_heads[h]
    wk_eff = ln_w[:, None] * Wk
    wv_eff = ln_w[:, None] * Wv

    def wq_layout(w):  # [1024,1024] -> [dqt, p, ko, m]
        return np.ascontiguousarray(
            w.reshape(8, 128, 8, 128).transpose(2, 1, 0, 3)
        ).astype(ml_dtypes.bfloat16)

    wq_a = wq_layout(wq_eff)
    wk_a = wq_layout(wk_eff)
    wv_a = np.ascontiguousarray(wv_eff.reshape(8, 128, DIM)).astype(
        ml_dtypes.bfloat16
    )
    wo_a = np.ascontiguousarray(Wo.reshape(8, 128, DIM)).astype(ml_dtypes.bfloat16)

    ones = np.ones((1, 128), np.float32)
    ident = np.eye(128, dtype=np.float32).astype(ml_dtypes.bfloat16)
    eyer = np.zeros((1, 256), np.float32)
    oh = np.zeros((16, 2048), np.float32)
    for h in range(NH):
        eyer[0, 16 * h + h] = 1.0
        oh[h, 128 * h:128 * (h + 1)] = 1.0

    # master[pj, plane, u]: r = u - pj - 2048 (= i_local - j_local)
    u = np.arange(MW, dtype=np.float64)[None, :]
    pj = np.arange(128, dtype=np.float64)[:, None]
    r = u - pj - 2048.0
    p0 = -np.abs(r)
    def _bf(a):
        return np.ascontiguousarray(
            np.maximum(a, -20000.0).astype(np.float32)
        ).astype(ml_dtypes.bfloat16)

    m0 = _bf(p0)
    masters1 = [_bf(-np.abs(r[:, :2048])), _bf(-np.abs(r[:, :2048] + 2048.0))]

    in_maps = []
    for c in range(8):
        b, t = c // 2, c % 2
        xr = np.ascontiguousarray(np.roll(x[b], -QTOK * t, axis=0)).astype(
            ml_dtypes.bfloat16
        )
        in_maps.append({
            "x": xr, "wq": wq_a, "wk": wk_a, "wv": wv_a, "wo": wo_a,
            "master": m0, "master1": masters1[t], "ones": ones, "ident": ident, "eyerow": eyer, "oh16": oh,
        })
    return s_heads, Ts, in_maps


def kernel(**inputs):
    global LAST_EXEC_NS
    s_heads, Ts, in_maps = _prep(**inputs)
    nc = _build_graph(s_heads, Ts)
    trace = os.environ.get("KERNEL_TRACE") == "1"
    res = run_bass_kernel_spmd(
        nc, in_maps, core_ids=list(range(8)), trace=trace
    )
    LAST_EXEC_NS = res.exec_time_ns
    out = np.empty((4, CTX, DIM), np.float32)
    for c in range(8):
        b, t = c // 2, c % 2
        out[b, QTOK * t:QTOK * (t + 1), :] = res.results[c]["out"]
    return out


# revision 38
# speedup vs baseline: 1.1873x; 1.0077x over previous
"""Distributed Bass kernel: LN + multi-head ALiBi attention + out-proj.

Sharding: 8 cores = (batch b in 0..3) x (query-token half t in 0..1).
Each core computes the full pipeline for its 1024 query tokens (all 16
heads); K/V are computed for the full 2048-token sequence (duplicated
across the 2 cores of a batch).  No collectives.

SPMD trick: every core runs the SAME graph.  Core (b, t) receives x[b]
rolled by -1024*t along tokens, so its query tokens always sit at local
rows 0..1023.  The ALiBi distance table ("master") is per-core DATA
encoding true global distances (two planes: j-tiles < 8 and >= 8, which
for t=1 differ by a 2048 wrap).  Blocks a core computes needlessly are
killed by the bias (exp(-large) ~ 0).

ALiBi banding: head h only effectively attends within |i-j| <= T_h =
ceil(23/s_h) (dropped softmax mass < ~1e-7 of the denominator), so
score blocks outside the band are skipped statically.

The per-head 1/(8*s_h) is folded into Wq on the host; the ALiBi bias is
added on DVE (PSUM + master slice), and ACT exp applies scale=s_h, so
exp(s_h*(QK/(8 s_h) + master)) is the softmax numerator.  The softmax
denominator arrives as a ones-column of the PV matmul; per q-chunk all
16 heads' sums are gathered into one PSUM tile via one-hot matmuls so a
SINGLE Ln serves the chunk (avoids ACT table-set thrashing), then a
per-head ones-matmul broadcast + exp(-x) forms 1/l across partitions
and OT is normalized in place.
"""

import os
import sys

sys.path.insert(0, "/opt/trn_rl_repo")

import numpy as np
import ml_dtypes

import concourse.bass as bass
import concourse.mybir as mybir
import concourse.tile as tile
from concourse import bacc
from concourse.bass import ts
from concourse.bass_utils import run_bass_kernel_spmd

BF16 = mybir.dt.bfloat16
F32 = mybir.dt.float32
F32R = mybir.dt.float32r

CTX = 2048
DIM = 1024
NH = 16
DH = 64
QTOK = 1024  # query tokens per core
EPS = 1e-5
MW = 3072  # master table width

LAST_EXEC_NS = None


def _band_blocks(T, qc):
    """j-tile list for query chunk qc (local g0 = qc*512), band half-width T."""
    g0 = qc * 512
    lo = max(0, g0 - T) // 128
    hi = (min(CTX, g0 + 512 + T) + 127) // 128
    jts = set(range(lo, hi))
    if qc == 0 and T < CTX:
        # wrap blocks: j_local in [2048-T, 2048) carries the left band of the
        # t=1 core (j_global ~ 1024-T..1024); bias-killed garbage for t=0.
        jts |= set(range((CTX - T) // 128, CTX // 128))
    return sorted(jts)


def _build_graph(s_heads, Ts):
    """Build the shared SPMD Bass graph; returns compiled nc."""
    nc = bacc.Bacc("TRN2", target_bir_lowering=False, debug=False)

    x_d = nc.dram_tensor("x", [CTX, DIM], BF16, kind="ExternalInput").ap()
    wq_d = nc.dram_tensor("wq", [8, 128, 8, 128], BF16, kind="ExternalInput").ap()
    wk_d = nc.dram_tensor("wk", [8, 128, 8, 128], BF16, kind="ExternalInput").ap()
    wv_d = nc.dram_tensor("wv", [8, 128, DIM], BF16, kind="ExternalInput").ap()
    wo_d = nc.dram_tensor("wo", [8, 128, DIM], BF16, kind="ExternalInput").ap()
    mst_d = nc.dram_tensor("master", [128, MW], BF16, kind="ExternalInput").ap()
    mst1_d = nc.dram_tensor("master1", [128, 2048], BF16, kind="ExternalInput").ap()
    one_d = nc.dram_tensor("ones", [1, 128], F32R, kind="ExternalInput").ap()
    idn_d = nc.dram_tensor("ident", [128, 128], BF16, kind="ExternalInput").ap()
    eye_d = nc.dram_tensor("eyerow", [1, 256], F32R, kind="ExternalInput").ap()
    oh_d = nc.dram_tensor("oh16", [16, 2048], F32R, kind="ExternalInput").ap()
    out_d = nc.dram_tensor("out", [QTOK, DIM], F32, kind="ExternalOutput").ap()

    AF = mybir.ActivationFunctionType
    ALU = mybir.AluOpType

    with tile.TileContext(nc) as tc:
        with (
            tc.tile_pool(name="persist", bufs=1) as pp,
            tc.tile_pool(name="dram", bufs=1, space="DRAM") as dp,
            tc.tile_pool(name="xio", bufs=2) as xp,
            tc.tile_pool(name="xnp", bufs=4) as xnp,
            tc.tile_pool(name="wstream", bufs=3) as wp,
            tc.tile_pool(name="ptile", bufs=2) as ptp,
            tc.tile_pool(name="small", bufs=4) as sp,
            tc.tile_pool(name="small2", bufs=2) as sp2,
            tc.tile_pool(name="norm", bufs=2) as epool,
            tc.tile_pool(name="wopool", bufs=1) as wop,
            tc.tile_pool(name="opool", bufs=2) as op,
            tc.tile_pool(name="ps_proj", bufs=2, space="PSUM") as ps_proj,
            tc.tile_pool(name="ps_s", bufs=3, space="PSUM") as ps_s,
            tc.tile_pool(name="ps_o", bufs=2, space="PSUM") as ps_o,
            tc.tile_pool(name="ps_n", bufs=1, space="PSUM") as ps_n,
        ):
            # ---- persistent SBUF ----
            master = pp.tile([128, MW], BF16, tag="master")
            master1 = pp.tile([128, 2048], BF16, tag="master1")
            ones = pp.tile([1, 128], F32R, tag="ones")
            ident = pp.tile([128, 128], BF16, tag="ident")
            eyer = pp.tile([1, 256], F32R, tag="eyer")
            oh16 = pp.tile([16, 2048], F32R, tag="oh16")
            xnT = pp.tile([128, 8, CTX], BF16, tag="big")  # slot shared with OT
            KT = pp.tile([128, 8, CTX], BF16, tag="KT")
            QT = pp.tile([128, 8, QTOK], BF16, tag="QT")
            Vsb = pp.tile([128, 16, NH, 65], BF16, tag="Vsb")
            wv_sb = pp.tile([128, 8, DIM], BF16, tag="wv")

            nc.scalar.dma_start(master[:], mst_d[:])
            nc.scalar.dma_start(master1[:], mst1_d[:])
            nc.scalar.dma_start(ones[:], one_d[:])
            nc.scalar.dma_start(ident[:], idn_d[:])
            nc.scalar.dma_start(eyer[:], eye_d[:])
            nc.scalar.dma_start(oh16[:], oh_d[:])
            eps_sb = pp.tile([128, 1], F32, tag="eps")
            nc.any.memset(eps_sb[:], EPS)
            nc.any.memset(Vsb[:, :, :, 64:65], 1.0)
            nc.scalar.dma_start(wv_sb[:], wv_d.rearrange("k p d -> p k d"))

            # ---- Phases A+B+C pipelined per 512-token chunk ----
            for ch in range(4):
                mv4 = sp.tile([128, 4, 2], F32, tag="mv4")
                for i, tt in enumerate(range(4 * ch, 4 * ch + 4)):
                    xt = xp.tile([128, DIM], BF16, tag="xt")
                    nc.sync.dma_start(xt[:], x_d[ts(tt, 128), :])
                    st6 = sp.tile([128, 2, 6], F32, tag="st6")
                    nc.vector.bn_stats(st6[:, 0, :], xt[:, 0:512])
                    nc.vector.bn_stats(st6[:, 1, :], xt[:, 512:1024])
                    nc.vector.bn_aggr(mv4[:, i, :], st6[:])
                # rstd for 4 tiles in one Ln + one Exp (keeps table set put)
                lv4 = sp.tile([128, 4], F32, tag="lv4")
                nc.scalar.activation(lv4[:], mv4[:, :, 1], AF.Ln, bias=eps_sb[:])
                rs4 = sp.tile([128, 4], F32, tag="rs4")
                nc.scalar.activation(rs4[:], lv4[:], AF.Exp, scale=-0.5)
                xns = []
                for i, tt in enumerate(range(4 * ch, 4 * ch + 4)):
                    xt = xp.tile([128, DIM], BF16, tag="xt")
                    nc.sync.dma_start(xt[:], x_d[ts(tt, 128), :])
                    xn = xnp.tile([128, DIM], BF16, tag="xn")
                    nc.vector.tensor_scalar(
                        xn[:], xt[:], mv4[:, i, 0:1], rs4[:, i:i + 1],
                        ALU.subtract, ALU.mult,
                    )
                    xns.append(xn)
                for ko in range(8):  # transpose on PE (proj PSUM pool reused)
                    tp = ps_proj.tile([128, 512], F32, tag="pp")
                    tpv = tp[:].bitcast(BF16)
                    for i in range(4):
                        nc.tensor.transpose(
                            tpv[:, ts(i, 128)], xns[i][:, ts(ko, 128)], ident[:]
                        )
                    nc.any.tensor_copy(xnT[:, ko, ts(ch, 512)], tpv[:, 0:512])
                for dqt in range(8):  # K^T chunk
                    wkt = wp.tile([128, 8, 128], BF16, tag="wqt")
                    nc.scalar.dma_start(wkt[:], wk_d[dqt])
                    ps = ps_proj.tile([128, 512], F32, tag="pp")
                    for ko in range(8):
                        nc.tensor.matmul(
                            ps[:], wkt[:, ko, :], xnT[:, ko, ts(ch, 512)],
                            start=(ko == 0), stop=(ko == 7),
                        )
                    nc.vector.tensor_copy(KT[:, dqt, ts(ch, 512)], ps[:])
                if ch < 2:
                    for dqt in range(8):  # Q^T chunk
                        wqt = wp.tile([128, 8, 128], BF16, tag="wqt")
                        nc.scalar.dma_start(wqt[:], wq_d[dqt])
                        ps = ps_proj.tile([128, 512], F32, tag="pp")
                        for ko in range(8):
                            nc.tensor.matmul(
                                ps[:], wqt[:, ko, :], xnT[:, ko, ts(ch, 512)],
                                start=(ko == 0), stop=(ko == 7),
                            )
                        nc.vector.tensor_copy(QT[:, dqt, ts(ch, 512)], ps[:])
                for jt in range(4 * ch, 4 * ch + 4):  # V chunk
                    for dvc in range(2):
                        ps = ps_proj.tile([128, 512], F32, tag="pp")
                        for ko in range(8):
                            nc.tensor.matmul(
                                ps[:], xnT[:, ko, ts(jt, 128)],
                                wv_sb[:, ko, ts(dvc, 512)],
                                start=(ko == 0), stop=(ko == 7),
                            )
                        nc.vector.tensor_copy(
                            Vsb[:, jt, dvc * 8:(dvc + 1) * 8, 0:64],
                            ps[:].rearrange("p (h d) -> p h d", d=64),
                        )

            # OT shares the xnT slot (all xnT readers are above)
            OT = pp.tile([128, 8, QTOK], BF16, tag="big")

            # ---- Phase D: banded attention (qc outer; batched softmax norm) --
            # Heads are processed in pairs occupying PE row-groups [0:64] and
            # [64:128]; their score matmuls run concurrently on the array.
            head_order = list(range(NH))
            head_groups = [(h,) for h in head_order]
            for qc in range(QTOK // 512):
                lcol = ps_n.tile([16, 512], F32, tag="lcol")
                for grp_heads in head_groups:
                    blocks = _band_blocks(max(Ts[h] for h in grp_heads), qc)
                    nb = len(blocks)
                    po = {}
                    for h in grp_heads:
                        po[h] = ps_o.tile([65, 512], F32, tag="po", name=f"po{h}")
                    for b0 in range(0, nb, 2):
                        grp = blocks[b0:b0 + 2]
                        g = len(grp)
                        ein = {}
                        pt = {}
                        for h in grp_heads:
                            ein[h] = ptp.tile([128, 1024], F32, tag="ein", name=f"ein{h}")
                            pt[h] = ptp.tile([128, 1024], BF16, tag="pt", name=f"pt{h}")
                        for gi, jt in enumerate(grp):
                            pss = {}
                            for h in grp_heads:
                                base = 64 * (h % 2)
                                dqt = h // 2
                                pss[h] = ps_s.tile([128, 512], F32, tag="ps", name=f"pss{h}")
                                nc.tensor.matmul(
                                    pss[h][:],
                                    KT[base:base + 64, dqt, ts(jt, 128)],
                                    QT[base:base + 64, dqt, ts(qc, 512)],
                                    start=True, stop=True,
                                )
                            off = 2048 + 512 * qc - 128 * jt
                            msrc = master1 if jt >= 8 else master
                            for h in grp_heads:
                                nc.vector.tensor_tensor(
                                    ein[h][:, ts(gi, 512)], pss[h][:],
                                    msrc[:, off:off + 512], ALU.add,
                                )
                        for h in grp_heads:
                            nc.scalar.activation(
                                pt[h][:, 0:g * 512], ein[h][:, 0:g * 512],
                                AF.Exp, scale=float(s_heads[h]),
                            )
                        for gi, jt in enumerate(grp):
                            for h in grp_heads:
                                nc.tensor.matmul(
                                    po[h][:], Vsb[:, jt, h, :],
                                    pt[h][:, ts(gi, 512)],
                                    start=(b0 + gi == 0),
                                    stop=(b0 + gi == nb - 1),
                                )
                    for h in grp_heads:
                        base = 64 * (h % 2)
                        dqt = h // 2
                        # gather this head's softmax sums into the shared lcol
                        lt = sp2.tile([1, 512], F32R, tag="ltmp")
                        nc.scalar.copy(lt[:], po[h][64:65, :])
                        nc.tensor.matmul(
                            lcol[:], eyer[0:1, ts(h, 16)], lt[:],
                            start=(h == head_order[0]),
                            stop=(h == head_order[-1]),
                        )
                        # stash unnormalized O^T
                        if base == 0:
                            nc.any.tensor_copy(
                                OT[0:64, dqt, ts(qc, 512)], po[h][0:64, :]
                            )
                        else:
                            tmp = epool.tile([64, 512], BF16, tag="otmp")
                            nc.any.tensor_copy(tmp[:], po[h][0:64, :])
                            nc.sync.dma_start(
                                OT[64:128, dqt, ts(qc, 512)], tmp[:]
                            )
                # batched softmax normalization: one Ln for all 16 heads
                lnl = sp2.tile([16, 512], F32R, tag="lnl16")
                nc.scalar.activation(lnl[:], lcol[:], AF.Ln)
                for j in range(8):  # head pair (2j, 2j+1) shares OT tile j
                    pb = ps_proj.tile([128, 512], F32, tag="pp", name=f"pb{j}")
                    nc.tensor.matmul(
                        pb[:], oh16[:, ts(j, 128)], lnl[:],
                        start=True, stop=True,
                    )
                    einv = epool.tile([128, 512], BF16, tag="einv")
                    nc.scalar.activation(einv[:], pb[:], AF.Exp, scale=-1.0)
                    nc.vector.tensor_tensor(
                        OT[:, j, ts(qc, 512)], OT[:, j, ts(qc, 512)],
                        einv[:], ALU.mult,
                    )
                # ---- output projection for this q-half (overlaps next qc) ----
                for ec in range(2):
                    wot = wop.tile([128, 8, 512], BF16, tag="wot")
                    nc.sync.dma_start(
                        wot[:], wo_d[:, :, ts(ec, 512)].rearrange("h p e -> p h e")
                    )
                    for it in range(4 * qc, 4 * qc + 4):
                        ps = ps_proj.tile([128, 512], F32, tag="pp")
                        for hdt in range(8):
                            nc.tensor.matmul(
                                ps[:], OT[:, hdt, ts(it, 128)], wot[:, hdt, :],
                                start=(hdt == 0), stop=(hdt == 7),
                            )
                        ot = op.tile([128, 512], F32, tag="ot")
                        nc.any.tensor_copy(ot[:], ps[:])
                        nc.sync.dma_start(out_d[ts(it, 128), ts(ec, 512)], ot[:])

    nc.compile()
    return nc


def _prep(x, ln_w, ln_b, Wq, Wk, Wv, Wo, M):
    """Host-side input preparation -> (s_heads, Ts, in_maps)."""
    x = np.asarray(x, np.float32)
    ln_w = np.asarray(ln_w, np.float32)
    ln_b = np.asarray(ln_b, np.float32)
    Wq = np.asarray(Wq, np.float32)
    Wk = np.asarray(Wk, np.float32)
    Wv = np.asarray(Wv, np.float32)
    Wo = np.asarray(Wo, np.float32)
    M = np.asarray(M, np.float32)
    assert not np.any(ln_b), "kernel assumes ln_b == 0 (folded LN bias unsupported)"

    s_heads = (-M[:, 0, 1]).astype(np.float64)  # M[h,0,1] = -s_h
    Ts = [min(CTX, int(np.ceil(19.0 / s))) for s in s_heads]

    wq_eff = ln_w[:, None] * Wq
    for h in range(NH):
        wq_eff[:, h * DH:(h + 1) * DH] /= 8.0 * s_heads[h]
    wk_eff = ln_w[:, None] * Wk
    wv_eff = ln_w[:, None] * Wv

    def wq_layout(w):  # [1024,1024] -> [dqt, p, ko, m]
        return np.ascontiguousarray(
            w.reshape(8, 128, 8, 128).transpose(2, 1, 0, 3)
        ).astype(ml_dtypes.bfloat16)

    wq_a = wq_layout(wq_eff)
    wk_a = wq_layout(wk_eff)
    wv_a = np.ascontiguousarray(wv_eff.reshape(8, 128, DIM)).astype(
        ml_dtypes.bfloat16
    )
    wo_a = np.ascontiguousarray(Wo.reshape(8, 128, DIM)).astype(ml_dtypes.bfloat16)

    ones = np.ones((1, 128), np.float32)
    ident = np.eye(128, dtype=np.float32).astype(ml_dtypes.bfloat16)
    eyer = np.zeros((1, 256), np.float32)
    oh = np.zeros((16, 2048), np.float32)
    for h in range(NH):
        eyer[0, 16 * h + h] = 1.0
    for j in range(8):
        oh[2 * j, 128 * j:128 * j + 64] = 1.0
        oh[2 * j + 1, 128 * j + 64:128 * (j + 1)] = 1.0

    # master[pj, plane, u]: r = u - pj - 2048 (= i_local - j_local)
    u = np.arange(MW, dtype=np.float64)[None, :]
    pj = np.arange(128, dtype=np.float64)[:, None]
    r = u - pj - 2048.0
    p0 = -np.abs(r)
    def _bf(a):
        return np.ascontiguousarray(
            np.maximum(a, -20000.0).astype(np.float32)
        ).astype(ml_dtypes.bfloat16)

    m0 = _bf(p0)
    masters1 = [_bf(-np.abs(r[:, :2048])), _bf(-np.abs(r[:, :2048] + 2048.0))]

    in_maps = []
    for c in range(8):
        b, t = c // 2, c % 2
        xr = np.ascontiguousarray(np.roll(x[b], -QTOK * t, axis=0)).astype(
            ml_dtypes.bfloat16
        )
        in_maps.append({
            "x": xr, "wq": wq_a, "wk": wk_a, "wv": wv_a, "wo": wo_a,
            "master": m0, "master1": masters1[t], "ones": ones, "ident": ident, "eyerow": eyer, "oh16": oh,
        })
    return s_heads, Ts, in_maps


def kernel(**inputs):
    global LAST_EXEC_NS
    s_heads, Ts, in_maps = _prep(**inputs)
    nc = _build_graph(s_heads, Ts)
    trace = os.environ.get("KERNEL_TRACE") == "1"
    res = run_bass_kernel_spmd(
        nc, in_maps, core_ids=list(range(8)), trace=trace
    )
    LAST_EXEC_NS = res.exec_time_ns
    out = np.empty((4, CTX, DIM), np.float32)
    for c in range(8):
        b, t = c // 2, c % 2
        out[b, QTOK * t:QTOK * (t + 1), :] = res.results[c]["out"]
    return out
